# revision 1
# baseline (speedup 1.0000x reference)
"""Paged-attention decode (GQA) on 8 Trainium2 NeuronCores.

Sharding: tensor-parallel over heads. Core c owns KV head c (KVH=8) and the
4 query heads in its GQA group. The KV cache is resolved, sliced per-core and
restaged by the host as bf16 (halving HBM read traffic vs f32), with the new
K/V token written in at position L-1 (so the device sees one uniform cache,
no separate new-token path). block_tables and seq_lens are folded into the
compiled graph (decode launch config). Each core runs an identical SPMD graph
with no collectives; the host concatenates the per-core output slices.

Host staging per core c (L = seq_lens[b], nt[b] = ceil(L/128) 128-token
tiles, concatenated across sequences; NBLK = sum(nt)):
  - kv [128, NBLK*256] bf16: per 128-token block i, cols [256i, 256i+128)
    hold K transposed (kv[d, 256i+t] = K[128i+t, d]) and cols
    [256i+128, 256i+256) hold V partition-major (kv[p, 256i+128+d]
    = V[128i+p, d]). K and V interleaved per block so one slab DMA
    (SLAB_BLOCKS*64KB, ~2MB) moves both at near-peak HBM bandwidth.
  - qh [128, B*G] bf16: queries as [d, (b, g)].

Device algorithm per core, per sequence b (tiles i = 0..nt-1):
  - DMA kv slabs (2MB contiguous-per-partition transfers, rotating pool)
  - scores[t, g] per tile: matmul(lhsT=KT_tile [d,t], rhs=q [d,4]) -> PSUM
  - exp(scale*s) on ACT (PSUM -> bf16 SBUF probs); mask tail rows of the
    last tile by a per-partition mask multiply (softmax-without-max:
    scores are O(5), no overflow)
  - out^T[d, 4] += matmul(lhsT=V_tile [t,d], rhs=probs tile [t,4]), PSUM acc
  - denominator l = ones-matmul over probs, reduced per sequence on DVE
  - finalize: broadcast 1/l via a rank-1 matmul, multiply, PE-transpose to
    [(b,g), d] layout, DMA out.
"""

import numpy as np
import sys

for _p in ("/opt/trn_rl_repo",):
    if _p not in sys.path:
        sys.path.append(_p)

SCALE = 0.08838834764831845
P = 128  # partition / head-dim / token-tile size


def _seq_order(L):
    """Processing order: big/small alternating. Every small sequence sits
    between two big ones, so its exp->PV dependency latency hides under a
    big sequence's QK phase; ends on the smallest for a short drain."""
    order0 = np.argsort(-np.asarray(L), kind="stable")
    B = len(order0)
    half = (B + 1) // 2
    order = np.empty(B, np.int64)
    order[0::2] = order0[:half]
    order[1::2] = order0[half:]
    return order


def _build_graph(
    nt,
    rem,
    nblk,
    fp8=True,
    dma_only=False,
    pipeline_pv=True,
    replay=1,
    no_dma=False,
    slab=None,
    bufs=10,
    pv_lag=1,
    altq=0,
    contend=False,
    qk2x=False,
    spbufs=3,
    pvint=True,
):
    """Build the SPMD Bacc graph, specialized on per-seq tile counts.

    nt[b]  = number of 128-token tiles for seq b (>= 1, includes new token)
    rem[b] = valid tokens in the last tile (1..128)
    nblk   = total 128-token blocks of the staged kv input (sum(nt))
    fp8    = stage K/V as float8 e3m4 (4 mantissa bits): halves HBM traffic
        again vs bf16. The PE multiplies fp8 stationary x bf16 moving
        exactly; q and probs stay bf16, so only the K/V quantization
        (~1.3e-2 rel) enters the error budget.
    dma_only = ablation: issue only the K/V loads (timing the memory floor)
    pipeline_pv = emit seq b's PV phase after seq b+1's score phase, so the
        exp round-trip through ScalarE doesn't stall the PE stream
    """
    import concourse.mybir as mybir
    import concourse.tile as tile
    from concourse import bacc
    from concourse.masks import make_identity

    B = len(nt)
    G = 4  # query heads per core
    MAXNT = int(max(nt))
    off = np.concatenate([[0], np.cumsum(np.asarray(nt, dtype=np.int64))])
    # kv slab boundaries (in 128-token blocks): small slabs at the start so
    # compute begins ~1us in, ~2MB steady-state slabs for peak HBM bandwidth,
    # small slabs at the end to shorten the post-DMA drain.
    SLAB = slab if slab else (64 if fp8 else 32)
    sizes = [max(SLAB // 8, 1), max(SLAB // 4, 1), max(SLAB // 2, 1)]
    while sum(sizes) < nblk - SLAB - sum((SLAB // 4, SLAB // 8)):
        sizes.append(SLAB)
    sizes += [max(SLAB // 4, 1), max(SLAB // 8, 1)]
    bounds = [0]
    for s in sizes:
        if bounds[-1] >= nblk:
            break
        bounds.append(min(nblk, bounds[-1] + s))
    if bounds[-1] < nblk:
        bounds.append(nblk)
    NS = len(bounds) - 1  # number of kv slabs
    slab_of = np.searchsorted(np.asarray(bounds), np.arange(nblk), side="right") - 1
    f32 = mybir.dt.float32
    bf16 = mybir.dt.bfloat16
    kvdt = mybir.dt.float8e3 if fp8 else bf16

    nc = bacc.Bacc(None, target_bir_lowering=False)
    kv = nc.dram_tensor("kv", [P, nblk * 2 * P], kvdt, kind="ExternalInput")
    qh = nc.dram_tensor("qh", [P, B * G], bf16, kind="ExternalInput")  # [d,(b,g)]
    out = nc.dram_tensor("out", [B, G * P], f32, kind="ExternalOutput")

    with tile.TileContext(nc) as tc:
        with tc.tile_pool(name="persist", bufs=1) as persist:
            ident_f = persist.tile([P, P], f32)
            make_identity(nc, ident_f)
            ones_col_bf = persist.tile([P, 1], bf16)
            nc.vector.memset(ones_col_bf, 1.0)
            ones_row_f = persist.tile([1, P], f32)
            nc.vector.memset(ones_row_f, 1.0)
            # neg_tab[p, r] = 0.0 if p < r else -30.0 — fused into the exp as
            # a per-partition bias for the partial last tile (r = rem):
            # exp(scale*s - 30) ~ 1e-13 zeroes the padding rows without a
            # separate DVE mask op in the PV dependency chain.
            neg_tab = persist.tile([P, P + 1], f32)
            nc.gpsimd.memset(neg_tab, -30.0)
            nc.gpsimd.affine_select(
                out=neg_tab,
                in_=neg_tab,
                compare_op=mybir.AluOpType.is_ge,
                fill=0.0,
                base=0,
                pattern=[[-1, P + 1]],
                channel_multiplier=1,
            )
            qh_bf = persist.tile([P, B * G], bf16)
            nc.gpsimd.dma_start(qh_bf[:], qh[:])
            outT = persist.tile([P, B * G], f32)  # [d, (b,g)]
            l_red = persist.tile([1, B * G], f32)
            recip = persist.tile([1, B * G], f32)
            outN = persist.tile([P, B * G], f32)
            outF = persist.tile([P, B * G], f32)

            if no_dma or contend:
                # no_dma=True: one resident dummy slab. no_dma=2: rotate
                # through 8 dummy-slab regions so PE stationary loads hit
                # spread SBUF addresses like the real kernel's rotating pool.
                n_dummy = 8 if no_dma == 2 else 1
                dummy_big = persist.tile([P, n_dummy * SLAB * 2 * P], kvdt)
                W = SLAB * 2 * P
                for j in range(n_dummy):
                    nc.vector.memset(dummy_big[:, j * W : (j + 1) * W], 0.0)
                dummies = [
                    dummy_big[:, j * W : (j + 1) * W] for j in range(n_dummy)
                ]

            with (
                tc.tile_pool(name="kv", bufs=bufs) as kvpool,
                tc.tile_pool(
                    name="sc_ps", bufs=spbufs or pv_lag + 1, space="PSUM"
                ) as scps,
                tc.tile_pool(name="probs", bufs=pv_lag + 1) as prpool,
                tc.tile_pool(name="acc_ps", bufs=3, space="PSUM") as accps,
            ):
                state = {}
                slabs = {}

                dma_engines = [nc.gpsimd, nc.sync, nc.scalar]

                def get_slab(s):
                    """Rotating-pool slab load; emission order is monotone in
                    s because blocks are visited in concat order."""
                    if no_dma or contend:
                        return dummies[s % len(dummies)]
                    if s not in slabs:
                        st = kvpool.tile([P, SLAB * 2 * P], kvdt)
                        lo = bounds[s] * 2 * P
                        hi = bounds[s + 1] * 2 * P
                        if altq == -1:
                            eng = nc.sync
                        elif altq:
                            eng = dma_engines[s % altq]
                        else:
                            eng = nc.gpsimd
                        eng.dma_start(st[:, : hi - lo], kv[:, lo:hi])
                        slabs[s] = st
                    return slabs[s]

                def _kt_of(i):  # K^T [d, t] of global block i
                    s = int(slab_of[i])
                    r = i - bounds[s]
                    return get_slab(s)[:, r * 2 * P : r * 2 * P + P]

                def _vt_of(i):  # V [t, d] of global block i
                    s = int(slab_of[i])
                    r = i - bounds[s]
                    return get_slab(s)[:, r * 2 * P + P : (r + 1) * 2 * P]

                def emit_scores(b, pv_cb=None):
                    ntb = int(nt[b])
                    o = int(off[b])
                    scores = scps.tile([P, G * MAXNT], f32)
                    for _rep in range(2 if qk2x else 1):
                        for i in range(ntb):
                            if (
                                pv_cb is not None
                                and i > 0
                                and slab_of[o + i] != slab_of[o + i - 1]
                            ):
                                # the PE is in-order: place the previous
                                # sequence's (ready) PV work ahead of the
                                # chunks that will wait on the next slab
                                pv_cb()
                                pv_cb = None
                            nc.tensor.matmul(
                                scores[:, G * i : G * (i + 1)],
                                lhsT=_kt_of(o + i),
                                rhs=qh_bf[:, G * b : G * (b + 1)],
                                start=True,
                                stop=True,
                            )
                    pb = prpool.tile([P, G * MAXNT], bf16)
                    r = int(rem[b])
                    full = G * (ntb - 1) if r < P else G * ntb
                    if full:
                        nc.scalar.activation(
                            pb[:, :full],
                            scores[:, :full],
                            mybir.ActivationFunctionType.Exp,
                            scale=SCALE,
                        )
                    if r < P:
                        nc.scalar.activation(
                            pb[:, G * (ntb - 1) : G * ntb],
                            scores[:, G * (ntb - 1) : G * ntb],
                            mybir.ActivationFunctionType.Exp,
                            bias=neg_tab[:, r : r + 1],
                            scale=SCALE,
                        )
                    state[b] = pb

                def emit_pv(b):
                    ntb = int(nt[b])
                    o = int(off[b])
                    pb = state.pop(b)
                    lp = accps.tile([1, G * MAXNT], f32, tag="acc")
                    nc.tensor.matmul(
                        lp[:, : G * ntb],
                        lhsT=ones_col_bf,
                        rhs=pb[:, : G * ntb],
                        start=True,
                        stop=True,
                    )
                    otp = accps.tile([P, G], f32, tag="acc")
                    for i in range(ntb):
                        nc.tensor.matmul(
                            otp,
                            lhsT=_vt_of(o + i),
                            rhs=pb[:, G * i : G * (i + 1)],
                            start=(i == 0),
                            stop=(i == ntb - 1),
                        )
                    nc.vector.tensor_copy(outT[:, G * b : G * (b + 1)], otp)
                    nc.vector.tensor_reduce(
                        l_red[0:1, G * b : G * (b + 1)],
                        lp[0:1, : G * ntb].rearrange("p (i h) -> p h i", h=G),
                        axis=mybir.AxisListType.X,
                        op=mybir.AluOpType.add,
                    )

                def emit_loads_raw():
                    """Real slab loads with tiny consumers (no compute dep)."""
                    for s in range(NS):
                        st = kvpool.tile([P, SLAB * 2 * P], kvdt)
                        lo = bounds[s] * 2 * P
                        hi = bounds[s + 1] * 2 * P
                        if altq == -1:
                            eng = nc.sync
                        elif altq:
                            eng = dma_engines[s % altq]
                        else:
                            eng = nc.gpsimd
                        eng.dma_start(st[:, : hi - lo], kv[:, lo:hi])
                        nc.vector.tensor_copy(outT[0:1, s : s + 1], st[0:1, 0:1])

                def emit_body():
                    slabs.clear()
                    if contend:
                        # real DMA stream + full compute on a dummy slab, no
                        # cross-deps: isolates resource contention from
                        # dependency stalls
                        emit_loads_raw()
                        pend = []
                        for b in range(B):
                            emit_scores(b)
                            pend.append(b)
                            if len(pend) > pv_lag:
                                emit_pv(pend.pop(0))
                        for b in pend:
                            emit_pv(b)
                    elif dma_only:
                        emit_loads_raw()
                        nc.vector.memset(l_red, 1.0)
                    elif pipeline_pv:
                        pend = []
                        for b in range(B):
                            cb = None
                            if pvint and len(pend) >= pv_lag:
                                cb = lambda: emit_pv(pend.pop(0))
                            emit_scores(b, cb)
                            pend.append(b)
                            if len(pend) > pv_lag:
                                emit_pv(pend.pop(0))
                        for b in pend:
                            emit_pv(b)
                    else:
                        for b in range(B):
                            emit_scores(b)
                            emit_pv(b)

                if replay > 1:
                    with tc.For_i(0, replay, 1):
                        emit_body()
                else:
                    emit_body()

            # ---- finalize: out = outT / l, transposed to [(b,g), d] ----
            with tc.tile_pool(name="fin_ps", bufs=1, space="PSUM") as finps:
                nc.vector.reciprocal(recip, l_red)
                bc = finps.tile([P, B * G], f32)
                nc.tensor.matmul(
                    bc, lhsT=ones_row_f, rhs=recip, start=True, stop=True
                )
                nc.vector.tensor_mul(outN, outT, bc)
                tp2 = finps.tile([P, B * G], f32)
                nc.tensor.transpose(tp2, outN, ident_f)
                nc.vector.tensor_copy(outF, tp2)
                nc.sync.dma_start(
                    out.rearrange("b (g d) -> (b g) d", g=G), outF
                )
    nc.compile()
    return nc


def _prepare(
    query,
    key,
    value,
    key_cache,
    value_cache,
    block_tables,
    seq_lens,
    build=True,
    fp8=True,
):
    """Build the compiled SPMD graph and the per-core input shards."""
    import ml_dtypes

    bf16 = ml_dtypes.bfloat16
    kvdt = ml_dtypes.float8_e3m4 if fp8 else bf16
    query = np.asarray(query, dtype=np.float32)
    key = np.asarray(key, dtype=np.float32)
    value = np.asarray(value, dtype=np.float32)
    key_cache = np.asarray(key_cache, dtype=np.float32)
    value_cache = np.asarray(value_cache, dtype=np.float32)
    block_tables = np.asarray(block_tables)
    seq_lens = np.asarray(seq_lens)

    B, H, D = query.shape
    KVH = key.shape[1]
    NB, BS = key_cache.shape[0], key_cache.shape[1]
    S_MAX = block_tables.shape[1] * BS
    G = H // KVH
    N_CORES = 8
    assert KVH == N_CORES and D == P

    L = np.maximum(seq_lens.astype(np.int64), 1)
    # `order[s]` = original index of the sequence processed s-th; outputs
    # are unscrambled on the host.
    order = _seq_order(L)
    L = L[order]
    nt = ((L + P - 1) // P).astype(np.int64)  # tiles incl. the new token
    rem = L - (nt - 1) * P  # valid tokens in last tile (1..128)
    off = np.concatenate([[0], np.cumsum(nt)])
    TOT = int(off[-1]) * P

    kc_flat = key_cache.reshape(NB * BS, KVH, D)
    vc_flat = value_cache.reshape(NB * BS, KVH, D)

    # Token slot ids, concatenated per sequence (nt[b]*128 tokens each; the
    # tail past L is read-but-masked padding). With arange block tables (the
    # spec's fill) slot (b, t) is just b*S_MAX + t.
    arange_ok = bool(
        np.array_equal(
            block_tables.ravel(),
            np.arange(block_tables.size, dtype=block_tables.ravel().dtype),
        )
    )
    tok_idx = np.empty(TOT, np.int64)
    for b in range(B):
        ob = int(order[b])  # original sequence index
        t = np.arange(int(nt[b]) * P, dtype=np.int64)
        # tile padding past the sequence's allocated pages re-reads the last
        # valid slot (finite data; zeroed by the exp mask anyway)
        t = np.minimum(t, S_MAX - 1)
        if arange_ok:
            ids = ob * S_MAX + t
        else:
            ids = block_tables[ob, t // BS].astype(np.int64) * BS + t % BS
        tok_idx[off[b] * P : (off[b] + nt[b]) * P] = ids
    newpos = off[:-1] * P + (L - 1)  # new token position in the concat layout

    NBLK = int(off[-1])
    nc = _build_graph(nt, rem, NBLK, fp8=fp8) if build else None

    lim = float(ml_dtypes.finfo(kvdt).max)
    in_maps = []
    for c in range(N_CORES):
        k_sel = kc_flat[tok_idx, c, :]  # [TOT, D] f32
        v_sel = vc_flat[tok_idx, c, :]
        k_sel[newpos] = key[order, c, :]
        v_sel[newpos] = value[order, c, :]
        kt3 = k_sel.T.reshape(P, NBLK, P)  # [d, blk, t]
        vp3 = v_sel.reshape(NBLK, P, P).transpose(1, 0, 2)  # [p, blk, d]
        kv_c = np.ascontiguousarray(
            np.stack([kt3, vp3], axis=2)
            .reshape(P, NBLK * 2 * P)
            .clip(-lim, lim)
            .astype(kvdt)
        )
        qh_c = np.ascontiguousarray(
            query[order][:, c * G : (c + 1) * G, :]
            .transpose(2, 0, 1)
            .reshape(D, B * G)
            .astype(bf16)
        )
        in_maps.append({"kv": kv_c, "qh": qh_c})
    return nc, in_maps, (B, H, D, G), order


def kernel(query, key, value, key_cache, value_cache, block_tables, seq_lens):
    from concourse.bass_utils import run_bass_kernel_spmd

    nc, in_maps, (B, H, D, G), order = _prepare(
        query, key, value, key_cache, value_cache, block_tables, seq_lens
    )
    res = run_bass_kernel_spmd(nc, in_maps, core_ids=list(range(len(in_maps))))
    out = np.empty((B, H * D), np.float32)
    for c in range(len(in_maps)):
        out[order, c * G * D : (c + 1) * G * D] = res.results[c]["out"]
    return out



# revision 17
# speedup vs baseline: 1.0939x; 1.0939x over previous
"""Paged-attention decode (GQA) on 8 Trainium2 NeuronCores.

Sharding: tensor-parallel over heads. Core c owns KV head c (KVH=8) and the
4 query heads in its GQA group. The KV cache is resolved, sliced per-core and
restaged by the host as bf16 (halving HBM read traffic vs f32), with the new
K/V token written in at position L-1 (so the device sees one uniform cache,
no separate new-token path). block_tables and seq_lens are folded into the
compiled graph (decode launch config). Each core runs an identical SPMD graph
with no collectives; the host concatenates the per-core output slices.

Host staging per core c (L = seq_lens[b], nt[b] = ceil(L/128) 128-token
tiles, concatenated across sequences; NBLK = sum(nt)):
  - kv [128, NBLK*256] bf16: per 128-token block i, cols [256i, 256i+128)
    hold K transposed (kv[d, 256i+t] = K[128i+t, d]) and cols
    [256i+128, 256i+256) hold V partition-major (kv[p, 256i+128+d]
    = V[128i+p, d]). K and V interleaved per block so one slab DMA
    (SLAB_BLOCKS*64KB, ~2MB) moves both at near-peak HBM bandwidth.
  - qh [128, B*G] bf16: queries as [d, (b, g)].

Device algorithm per core, per sequence b (tiles i = 0..nt-1):
  - DMA kv slabs (2MB contiguous-per-partition transfers, rotating pool)
  - scores[t, g] per tile: matmul(lhsT=KT_tile [d,t], rhs=q [d,4]) -> PSUM
  - exp(scale*s) on ACT (PSUM -> bf16 SBUF probs); mask tail rows of the
    last tile by a per-partition mask multiply (softmax-without-max:
    scores are O(5), no overflow)
  - out^T[d, 4] += matmul(lhsT=V_tile [t,d], rhs=probs tile [t,4]), PSUM acc
  - denominator l = ones-matmul over probs, reduced per sequence on DVE
  - finalize: broadcast 1/l via a rank-1 matmul, multiply, PE-transpose to
    [(b,g), d] layout, DMA out.
"""

import numpy as np
import sys

for _p in ("/opt/trn_rl_repo",):
    if _p not in sys.path:
        sys.path.append(_p)

SCALE = 0.08838834764831845
P = 128  # partition / head-dim / token-tile size


def _seq_order(L):
    """Processing order: big/small alternating. Every small sequence sits
    between two big ones, so its exp->PV dependency latency hides under a
    big sequence's QK phase; ends on the smallest for a short drain."""
    order0 = np.argsort(-np.asarray(L), kind="stable")
    B = len(order0)
    half = (B + 1) // 2
    order = np.empty(B, np.int64)
    order[0::2] = order0[:half]
    order[1::2] = order0[half:]
    return order


def _build_graph(
    nt,
    rem,
    nblk,
    fp8=True,
    dma_only=False,
    pipeline_pv=True,
    replay=1,
    no_dma=False,
    slab=None,
    bufs=10,
    pv_lag=1,
    altq=-1,
    contend=False,
    qk2x=False,
    spbufs=3,
    pvint=True,
    ramp=None,
    fin_chunks=2,
):
    """Build the SPMD Bacc graph, specialized on per-seq tile counts.

    nt[b]  = number of 128-token tiles for seq b (>= 1, includes new token)
    rem[b] = valid tokens in the last tile (1..128)
    nblk   = total 128-token blocks of the staged kv input (sum(nt))
    fp8    = stage K/V as float8 e3m4 (4 mantissa bits): halves HBM traffic
        again vs bf16. The PE multiplies fp8 stationary x bf16 moving
        exactly; q and probs stay bf16, so only the K/V quantization
        (~1.3e-2 rel) enters the error budget.
    dma_only = ablation: issue only the K/V loads (timing the memory floor)
    pipeline_pv = emit seq b's PV phase after seq b+1's score phase, so the
        exp round-trip through ScalarE doesn't stall the PE stream
    """
    import concourse.mybir as mybir
    import concourse.tile as tile
    from concourse import bacc
    from concourse.masks import make_identity

    B = len(nt)
    G = 4  # query heads per core
    MAXNT = int(max(nt))
    off = np.concatenate([[0], np.cumsum(np.asarray(nt, dtype=np.int64))])
    # kv slab boundaries (in 128-token blocks): small slabs at the start so
    # compute begins ~1us in, ~2MB steady-state slabs for peak HBM bandwidth,
    # small slabs at the end to shorten the post-DMA drain.
    SLAB = slab if slab else (64 if fp8 else 32)
    if ramp is not None:
        up, down = [list(r) for r in ramp]
    else:
        up = [max(SLAB // 8, 1), max(SLAB // 4, 1), max(SLAB // 2, 1)]
        down = [max(SLAB // 4, 1), max(SLAB // 8, 1), max(SLAB // 16, 2)]
    mid = nblk - sum(up) - sum(down)
    if mid <= 0:
        sizes = up  # tiny problem: ramp-up only
    else:
        sizes = up + [SLAB] * (mid // SLAB)
        if mid % SLAB:
            sizes.append(mid % SLAB)  # odd slab just before the ramp-down
        sizes += down
    bounds = [0]
    for s in sizes:
        if bounds[-1] >= nblk:
            break
        bounds.append(min(nblk, bounds[-1] + s))
    if bounds[-1] < nblk:
        bounds.append(nblk)
    NS = len(bounds) - 1  # number of kv slabs
    slab_of = np.searchsorted(np.asarray(bounds), np.arange(nblk), side="right") - 1
    f32 = mybir.dt.float32
    bf16 = mybir.dt.bfloat16
    kvdt = mybir.dt.float8e3 if fp8 else bf16

    nc = bacc.Bacc(None, target_bir_lowering=False)
    kv = nc.dram_tensor("kv", [P, nblk * 2 * P], kvdt, kind="ExternalInput")
    qh = nc.dram_tensor("qh", [P, B * G], bf16, kind="ExternalInput")  # [d,(b,g)]
    out = nc.dram_tensor("out", [B, G * P], f32, kind="ExternalOutput")

    with tile.TileContext(nc) as tc:
        with tc.tile_pool(name="persist", bufs=1) as persist:
            ident_f = persist.tile([P, P], f32)
            make_identity(nc, ident_f)
            ones_col_bf = persist.tile([P, 1], bf16)
            nc.vector.memset(ones_col_bf, 1.0)
            ones_row_f = persist.tile([1, P], f32)
            nc.vector.memset(ones_row_f, 1.0)
            # neg_tab[p, r] = 0.0 if p < r else -30.0 — fused into the exp as
            # a per-partition bias for the partial last tile (r = rem):
            # exp(scale*s - 30) ~ 1e-13 zeroes the padding rows without a
            # separate DVE mask op in the PV dependency chain.
            neg_tab = persist.tile([P, P + 1], f32)
            nc.gpsimd.memset(neg_tab, -30.0)
            nc.gpsimd.affine_select(
                out=neg_tab,
                in_=neg_tab,
                compare_op=mybir.AluOpType.is_ge,
                fill=0.0,
                base=0,
                pattern=[[-1, P + 1]],
                channel_multiplier=1,
            )
            qh_bf = persist.tile([P, B * G], bf16)
            nc.gpsimd.dma_start(qh_bf[:], qh[:])
            outT = persist.tile([P, B * G], f32)  # [d, (b,g)]
            l_red = persist.tile([1, B * G], f32)
            recip = persist.tile([1, B * G], f32)

            if no_dma or contend:
                # no_dma=True: one resident dummy slab. no_dma=2: rotate
                # through 8 dummy-slab regions so PE stationary loads hit
                # spread SBUF addresses like the real kernel's rotating pool.
                n_dummy = 8 if no_dma == 2 else 1
                dummy_big = persist.tile([P, n_dummy * SLAB * 2 * P], kvdt)
                W = SLAB * 2 * P
                for j in range(n_dummy):
                    nc.vector.memset(dummy_big[:, j * W : (j + 1) * W], 0.0)
                dummies = [
                    dummy_big[:, j * W : (j + 1) * W] for j in range(n_dummy)
                ]

            with (
                tc.tile_pool(name="kv", bufs=bufs) as kvpool,
                tc.tile_pool(
                    name="sc_ps", bufs=spbufs or pv_lag + 1, space="PSUM"
                ) as scps,
                tc.tile_pool(name="probs", bufs=pv_lag + 1) as prpool,
                tc.tile_pool(name="acc_ps", bufs=3, space="PSUM") as accps,
                tc.tile_pool(name="fin_ps", bufs=1, space="PSUM") as finps,
                tc.tile_pool(name="fin_sb", bufs=2) as fpool,
            ):
                state = {}
                slabs = {}

                dma_engines = [nc.gpsimd, nc.sync, nc.scalar]

                def get_slab(s):
                    """Rotating-pool slab load; emission order is monotone in
                    s because blocks are visited in concat order."""
                    if no_dma or contend:
                        return dummies[s % len(dummies)]
                    if s not in slabs:
                        st = kvpool.tile([P, SLAB * 2 * P], kvdt)
                        lo = bounds[s] * 2 * P
                        hi = bounds[s + 1] * 2 * P
                        if altq == -1:
                            eng = nc.sync
                        elif altq:
                            eng = dma_engines[s % altq]
                        else:
                            eng = nc.gpsimd
                        eng.dma_start(st[:, : hi - lo], kv[:, lo:hi])
                        slabs[s] = st
                    return slabs[s]

                def _kt_of(i):  # K^T [d, t] of global block i
                    s = int(slab_of[i])
                    r = i - bounds[s]
                    return get_slab(s)[:, r * 2 * P : r * 2 * P + P]

                def _vt_of(i):  # V [t, d] of global block i
                    s = int(slab_of[i])
                    r = i - bounds[s]
                    return get_slab(s)[:, r * 2 * P + P : (r + 1) * 2 * P]

                def emit_scores(b, pv_cb=None):
                    ntb = int(nt[b])
                    o = int(off[b])
                    scores = scps.tile([P, G * MAXNT], f32)
                    for _rep in range(2 if qk2x else 1):
                        for i in range(ntb):
                            if (
                                pv_cb is not None
                                and i > 0
                                and slab_of[o + i] != slab_of[o + i - 1]
                            ):
                                # the PE is in-order: place the previous
                                # sequence's (ready) PV work ahead of the
                                # chunks that will wait on the next slab
                                pv_cb()
                                pv_cb = None
                            nc.tensor.matmul(
                                scores[:, G * i : G * (i + 1)],
                                lhsT=_kt_of(o + i),
                                rhs=qh_bf[:, G * b : G * (b + 1)],
                                start=True,
                                stop=True,
                            )
                    pb = prpool.tile([P, G * MAXNT], bf16)
                    r = int(rem[b])
                    full = G * (ntb - 1) if r < P else G * ntb
                    if full:
                        nc.scalar.activation(
                            pb[:, :full],
                            scores[:, :full],
                            mybir.ActivationFunctionType.Exp,
                            scale=SCALE,
                        )
                    if r < P:
                        nc.scalar.activation(
                            pb[:, G * (ntb - 1) : G * ntb],
                            scores[:, G * (ntb - 1) : G * ntb],
                            mybir.ActivationFunctionType.Exp,
                            bias=neg_tab[:, r : r + 1],
                            scale=SCALE,
                        )
                    state[b] = pb

                def emit_pv(b):
                    ntb = int(nt[b])
                    o = int(off[b])
                    pb = state.pop(b)
                    lp = accps.tile([1, G * MAXNT], f32, tag="acc")
                    nc.tensor.matmul(
                        lp[:, : G * ntb],
                        lhsT=ones_col_bf,
                        rhs=pb[:, : G * ntb],
                        start=True,
                        stop=True,
                    )
                    otp = accps.tile([P, G], f32, tag="acc")
                    for i in range(ntb):
                        nc.tensor.matmul(
                            otp,
                            lhsT=_vt_of(o + i),
                            rhs=pb[:, G * i : G * (i + 1)],
                            start=(i == 0),
                            stop=(i == ntb - 1),
                        )
                    nc.vector.tensor_copy(outT[:, G * b : G * (b + 1)], otp)
                    nc.vector.tensor_reduce(
                        l_red[0:1, G * b : G * (b + 1)],
                        lp[0:1, : G * ntb].rearrange("p (i h) -> p h i", h=G),
                        axis=mybir.AxisListType.X,
                        op=mybir.AluOpType.add,
                    )

                out_r = out.rearrange("b (g d) -> (b g) d", g=G)

                def emit_finalize(s0, s1):
                    """Normalize + transpose + store seqs [s0, s1): all but the
                    last chunk hides under the ongoing DMA/PE stream."""
                    c0, c1 = G * s0, G * s1
                    n = c1 - c0
                    nc.vector.reciprocal(recip[0:1, c0:c1], l_red[0:1, c0:c1])
                    bc = finps.tile([P, P], f32)
                    nc.tensor.matmul(
                        bc[:, :n],
                        lhsT=ones_row_f,
                        rhs=recip[0:1, c0:c1],
                        start=True,
                        stop=True,
                    )
                    outN = fpool.tile([P, P], f32)
                    nc.vector.tensor_mul(outN[:, :n], outT[:, c0:c1], bc[:, :n])
                    tp2 = finps.tile([P, P], f32)
                    nc.tensor.transpose(tp2[:n, :], outN[:, :n], ident_f)
                    outF = fpool.tile([P, P], f32)
                    nc.vector.tensor_copy(outF[:n, :], tp2[:n, :])
                    nc.sync.dma_start(out_r[c0:c1, :], outF[:n, :])

                def emit_loads_raw():
                    """Real slab loads with tiny consumers (no compute dep)."""
                    for s in range(NS):
                        st = kvpool.tile([P, SLAB * 2 * P], kvdt)
                        lo = bounds[s] * 2 * P
                        hi = bounds[s + 1] * 2 * P
                        if altq == -1:
                            eng = nc.sync
                        elif altq:
                            eng = dma_engines[s % altq]
                        else:
                            eng = nc.gpsimd
                        eng.dma_start(st[:, : hi - lo], kv[:, lo:hi])
                        nc.vector.tensor_copy(outT[0:1, s : s + 1], st[0:1, 0:1])

                def emit_body():
                    slabs.clear()
                    fbs = (
                        [B]
                        if fin_chunks <= 1
                        else [B * (i + 1) // fin_chunks for i in range(fin_chunks)]
                    )
                    done, prev = [0], [0]

                    def run_pv(b):
                        emit_pv(b)
                        done[0] += 1
                        while fbs and done[0] >= fbs[0]:
                            s1 = fbs.pop(0)
                            emit_finalize(prev[0], s1)
                            prev[0] = s1

                    if contend:
                        # real DMA stream + full compute on a dummy slab, no
                        # cross-deps: isolates resource contention from
                        # dependency stalls
                        emit_loads_raw()
                        pend = []
                        for b in range(B):
                            emit_scores(b)
                            pend.append(b)
                            if len(pend) > pv_lag:
                                run_pv(pend.pop(0))
                        for b in pend:
                            run_pv(b)
                    elif dma_only:
                        emit_loads_raw()
                        nc.vector.memset(l_red, 1.0)
                        nc.vector.memset(outT, 0.0)
                        emit_finalize(0, B)
                    elif pipeline_pv:
                        pend = []
                        for b in range(B):
                            cb = None
                            if pvint and len(pend) >= pv_lag:
                                cb = lambda: run_pv(pend.pop(0))
                            emit_scores(b, cb)
                            pend.append(b)
                            if len(pend) > pv_lag:
                                run_pv(pend.pop(0))
                        for b in pend:
                            run_pv(b)
                    else:
                        for b in range(B):
                            emit_scores(b)
                            run_pv(b)

                if replay > 1:
                    with tc.For_i(0, replay, 1):
                        emit_body()
                else:
                    emit_body()
    nc.compile()
    return nc


def _build_probe(nt, rem, nblk, probe, replay=1, slab=64, bufs=6, n_dummy=4):
    """Contention probes with ZERO shared tiles between the DMA stream and the
    compute stream (the old `contend` mode shared outT and the DVE queue,
    serializing the two streams through Tile dependencies).

    probe: 'dma'    = slab loads only, consumers on Pool engine
           'comp'   = full compute mix (QK+exp+PV+DVE) on dummy slabs
           'qk'     = pure PE stream (QK+PV matmuls, dummy probs, no ACT/DVE)
           'both'   = 'dma' + 'comp' concurrently, disjoint resources
           'qk_dma' = 'dma' + 'qk' concurrently, disjoint resources
    """
    import concourse.mybir as mybir
    import concourse.tile as tile
    from concourse import bacc

    B = len(nt)
    G = 4
    MAXNT = int(max(nt))
    off = np.concatenate([[0], np.cumsum(np.asarray(nt, dtype=np.int64))])
    SLAB = slab
    bounds = list(range(0, nblk, SLAB)) + [nblk]
    if bounds[-2] == nblk:
        bounds = bounds[:-1]
    NS = len(bounds) - 1
    f32 = mybir.dt.float32
    bf16 = mybir.dt.bfloat16
    kvdt = mybir.dt.float8e3

    nc = bacc.Bacc(None, target_bir_lowering=False)
    kv = nc.dram_tensor("kv", [P, nblk * 2 * P], kvdt, kind="ExternalInput")
    qh = nc.dram_tensor("qh", [P, B * G], bf16, kind="ExternalInput")
    out = nc.dram_tensor("out", [B, G * P], f32, kind="ExternalOutput")

    do_dma = probe in ("dma", "both", "qk_dma", "qk1_dma", "qk2_dma")
    do_comp = probe in ("comp", "both")
    do_qk = probe in ("qk", "qk_dma")
    do_qk1 = probe in ("qk1", "qk1_dma")  # QK matmuls only (half PE work)
    do_qk2 = probe in ("qk2", "qk2_dma")  # QK emitted twice (double PE work)

    with tile.TileContext(nc) as tc:
        with tc.tile_pool(name="persist", bufs=1) as persist:
            qh_bf = persist.tile([P, B * G], bf16)
            nc.gpsimd.dma_start(qh_bf[:], qh[:])
            outz = persist.tile([B, G * P], f32)
            nc.vector.memset(outz, 0.0)
            sink = persist.tile([1, NS + 1], kvdt)  # Pool consumer target
            neg_tab = persist.tile([P, P + 1], f32)
            nc.gpsimd.memset(neg_tab, -30.0)
            W = SLAB * 2 * P
            dummies = []
            if do_comp or do_qk or do_qk1 or do_qk2:
                dummy_big = persist.tile([P, n_dummy * W], kvdt)
                for j in range(n_dummy):
                    nc.vector.memset(dummy_big[:, j * W : (j + 1) * W], 0.0)
                dummies = [dummy_big[:, j * W : (j + 1) * W] for j in range(n_dummy)]
            pdum = None
            if do_qk or do_qk1 or do_qk2:
                pdum = persist.tile([P, G * MAXNT], bf16)
                nc.vector.memset(pdum, 0.001)
            outT = persist.tile([P, B * G], f32)
            l_red = persist.tile([1, B * G], f32)
            ones_col_bf = persist.tile([P, 1], bf16)
            nc.vector.memset(ones_col_bf, 1.0)

            with (
                tc.tile_pool(name="kv", bufs=bufs) as kvpool,
                tc.tile_pool(name="sc_ps", bufs=3, space="PSUM") as scps,
                tc.tile_pool(name="probs", bufs=2) as prpool,
                tc.tile_pool(name="acc_ps", bufs=3, space="PSUM") as accps,
            ):
                def emit_loads():
                    for s in range(NS):
                        st = kvpool.tile([P, SLAB * 2 * P], kvdt)
                        lo, hi = bounds[s] * 2 * P, bounds[s + 1] * 2 * P
                        nc.sync.dma_start(st[:, : hi - lo], kv[:, lo:hi])
                        nc.gpsimd.tensor_copy(sink[0:1, s : s + 1], st[0:1, 0:1])

                def _kt(i):
                    s, r = divmod(int(i), SLAB)
                    d = dummies[s % n_dummy]
                    return d[:, (r % SLAB) * 2 * P : (r % SLAB) * 2 * P + P]

                def _vt(i):
                    s, r = divmod(int(i), SLAB)
                    d = dummies[s % n_dummy]
                    return d[:, (r % SLAB) * 2 * P + P : ((r % SLAB) + 1) * 2 * P]

                def emit_compute(full):
                    state = {}
                    pend = []
                    for b in range(B):
                        ntb, o = int(nt[b]), int(off[b])
                        scores = scps.tile([P, G * MAXNT], f32)
                        for i in range(ntb):
                            nc.tensor.matmul(
                                scores[:, G * i : G * (i + 1)],
                                lhsT=_kt(o + i),
                                rhs=qh_bf[:, G * b : G * (b + 1)],
                                start=True, stop=True,
                            )
                        if full:
                            pb = prpool.tile([P, G * MAXNT], bf16)
                            nc.scalar.activation(
                                pb[:, : G * ntb], scores[:, : G * ntb],
                                mybir.ActivationFunctionType.Exp, scale=SCALE,
                            )
                            state[b] = pb
                        pend.append(b)
                        if len(pend) > 1:
                            bb = pend.pop(0)
                            nb2, o2 = int(nt[bb]), int(off[bb])
                            pbb = state.pop(bb) if full else pdum
                            otp = accps.tile([P, G], f32, tag="acc")
                            for i in range(nb2):
                                nc.tensor.matmul(
                                    otp, lhsT=_vt(o2 + i),
                                    rhs=pbb[:, G * i : G * (i + 1)],
                                    start=(i == 0), stop=(i == nb2 - 1),
                                )
                            if full:
                                lp = accps.tile([1, G * MAXNT], f32, tag="acc")
                                nc.tensor.matmul(
                                    lp[:, : G * nb2], lhsT=ones_col_bf,
                                    rhs=pbb[:, : G * nb2], start=True, stop=True,
                                )
                                nc.vector.tensor_copy(outT[:, G * bb : G * (bb + 1)], otp)
                                nc.vector.tensor_reduce(
                                    l_red[0:1, G * bb : G * (bb + 1)],
                                    lp[0:1, : G * nb2].rearrange("p (i h) -> p h i", h=G),
                                    axis=mybir.AxisListType.X,
                                    op=mybir.AluOpType.add,
                                )
                            else:
                                nc.vector.tensor_copy(outT[:, G * bb : G * (bb + 1)], otp)
                    for bb in pend:
                        nb2, o2 = int(nt[bb]), int(off[bb])
                        pbb = state.pop(bb) if full else pdum
                        otp = accps.tile([P, G], f32, tag="acc")
                        for i in range(nb2):
                            nc.tensor.matmul(
                                otp, lhsT=_vt(o2 + i),
                                rhs=pbb[:, G * i : G * (i + 1)],
                                start=(i == 0), stop=(i == nb2 - 1),
                            )
                        nc.vector.tensor_copy(outT[:, G * bb : G * (bb + 1)], otp)

                def emit_qk_only():
                    for b in range(B):
                        ntb, o = int(nt[b]), int(off[b])
                        scores = scps.tile([P, G * MAXNT], f32)
                        for i in range(ntb):
                            nc.tensor.matmul(
                                scores[:, G * i : G * (i + 1)],
                                lhsT=_kt(o + i),
                                rhs=qh_bf[:, G * b : G * (b + 1)],
                                start=True, stop=True,
                            )

                def body():
                    if do_dma:
                        emit_loads()
                    if do_comp:
                        emit_compute(full=True)
                    if do_qk:
                        emit_compute(full=False)
                    if do_qk1:
                        emit_qk_only()
                    if do_qk2:
                        emit_compute(full=False)
                        emit_compute(full=False)

                if replay > 1:
                    with tc.For_i(0, replay, 1):
                        body()
                else:
                    body()

            nc.sync.dma_start(out[:, :], outz)
    nc.compile()
    return nc


def _prepare(
    query,
    key,
    value,
    key_cache,
    value_cache,
    block_tables,
    seq_lens,
    build=True,
    fp8=True,
):
    """Build the compiled SPMD graph and the per-core input shards."""
    import ml_dtypes

    bf16 = ml_dtypes.bfloat16
    kvdt = ml_dtypes.float8_e3m4 if fp8 else bf16
    query = np.asarray(query, dtype=np.float32)
    key = np.asarray(key, dtype=np.float32)
    value = np.asarray(value, dtype=np.float32)
    key_cache = np.asarray(key_cache, dtype=np.float32)
    value_cache = np.asarray(value_cache, dtype=np.float32)
    block_tables = np.asarray(block_tables)
    seq_lens = np.asarray(seq_lens)

    B, H, D = query.shape
    KVH = key.shape[1]
    NB, BS = key_cache.shape[0], key_cache.shape[1]
    S_MAX = block_tables.shape[1] * BS
    G = H // KVH
    N_CORES = 8
    assert KVH == N_CORES and D == P

    L = np.maximum(seq_lens.astype(np.int64), 1)
    # `order[s]` = original index of the sequence processed s-th; outputs
    # are unscrambled on the host.
    order = _seq_order(L)
    L = L[order]
    nt = ((L + P - 1) // P).astype(np.int64)  # tiles incl. the new token
    rem = L - (nt - 1) * P  # valid tokens in last tile (1..128)
    off = np.concatenate([[0], np.cumsum(nt)])
    TOT = int(off[-1]) * P

    kc_flat = key_cache.reshape(NB * BS, KVH, D)
    vc_flat = value_cache.reshape(NB * BS, KVH, D)

    # Token slot ids, concatenated per sequence (nt[b]*128 tokens each; the
    # tail past L is read-but-masked padding). With arange block tables (the
    # spec's fill) slot (b, t) is just b*S_MAX + t.
    arange_ok = bool(
        np.array_equal(
            block_tables.ravel(),
            np.arange(block_tables.size, dtype=block_tables.ravel().dtype),
        )
    )
    tok_idx = np.empty(TOT, np.int64)
    for b in range(B):
        ob = int(order[b])  # original sequence index
        t = np.arange(int(nt[b]) * P, dtype=np.int64)
        # tile padding past the sequence's allocated pages re-reads the last
        # valid slot (finite data; zeroed by the exp mask anyway)
        t = np.minimum(t, S_MAX - 1)
        if arange_ok:
            ids = ob * S_MAX + t
        else:
            ids = block_tables[ob, t // BS].astype(np.int64) * BS + t % BS
        tok_idx[off[b] * P : (off[b] + nt[b]) * P] = ids
    newpos = off[:-1] * P + (L - 1)  # new token position in the concat layout

    NBLK = int(off[-1])
    nc = _build_graph(nt, rem, NBLK, fp8=fp8) if build else None

    lim = float(ml_dtypes.finfo(kvdt).max)
    in_maps = []
    for c in range(N_CORES):
        k_sel = kc_flat[tok_idx, c, :]  # [TOT, D] f32
        v_sel = vc_flat[tok_idx, c, :]
        k_sel[newpos] = key[order, c, :]
        v_sel[newpos] = value[order, c, :]
        kt3 = k_sel.T.reshape(P, NBLK, P)  # [d, blk, t]
        vp3 = v_sel.reshape(NBLK, P, P).transpose(1, 0, 2)  # [p, blk, d]
        kv_c = np.ascontiguousarray(
            np.stack([kt3, vp3], axis=2)
            .reshape(P, NBLK * 2 * P)
            .clip(-lim, lim)
            .astype(kvdt)
        )
        qh_c = np.ascontiguousarray(
            query[order][:, c * G : (c + 1) * G, :]
            .transpose(2, 0, 1)
            .reshape(D, B * G)
            .astype(bf16)
        )
        in_maps.append({"kv": kv_c, "qh": qh_c})
    return nc, in_maps, (B, H, D, G), order


def kernel(query, key, value, key_cache, value_cache, block_tables, seq_lens):
    from concourse.bass_utils import run_bass_kernel_spmd

    nc, in_maps, (B, H, D, G), order = _prepare(
        query, key, value, key_cache, value_cache, block_tables, seq_lens
    )
    res = run_bass_kernel_spmd(nc, in_maps, core_ids=list(range(len(in_maps))))
    out = np.empty((B, H * D), np.float32)
    for c in range(len(in_maps)):
        out[order, c * G * D : (c + 1) * G * D] = res.results[c]["out"]
    return out



# revision 19
# speedup vs baseline: 1.1440x; 1.0459x over previous
"""Paged-attention decode (GQA) on 8 Trainium2 NeuronCores.

Sharding: tensor-parallel over heads. Core c owns KV head c (KVH=8) and the
4 query heads in its GQA group. The KV cache is resolved, sliced per-core and
restaged by the host as bf16 (halving HBM read traffic vs f32), with the new
K/V token written in at position L-1 (so the device sees one uniform cache,
no separate new-token path). block_tables and seq_lens are folded into the
compiled graph (decode launch config). Each core runs an identical SPMD graph
with no collectives; the host concatenates the per-core output slices.

Host staging per core c (L = seq_lens[b], nt[b] = ceil(L/128) 128-token
tiles, concatenated across sequences; NBLK = sum(nt)):
  - kv [128, NBLK*256] bf16: per 128-token block i, cols [256i, 256i+128)
    hold K transposed (kv[d, 256i+t] = K[128i+t, d]) and cols
    [256i+128, 256i+256) hold V partition-major (kv[p, 256i+128+d]
    = V[128i+p, d]). K and V interleaved per block so one slab DMA
    (SLAB_BLOCKS*64KB, ~2MB) moves both at near-peak HBM bandwidth.
  - qh [128, B*G] bf16: queries as [d, (b, g)].

Device algorithm per core, per sequence b (tiles i = 0..nt-1):
  - DMA kv slabs (2MB contiguous-per-partition transfers, rotating pool)
  - scores[t, g] per tile: matmul(lhsT=KT_tile [d,t], rhs=q [d,4]) -> PSUM
  - exp(scale*s) on ACT (PSUM -> bf16 SBUF probs); mask tail rows of the
    last tile by a per-partition mask multiply (softmax-without-max:
    scores are O(5), no overflow)
  - out^T[d, 4] += matmul(lhsT=V_tile [t,d], rhs=probs tile [t,4]), PSUM acc
  - denominator l = ones-matmul over probs, reduced per sequence on DVE
  - finalize: broadcast 1/l via a rank-1 matmul, multiply, PE-transpose to
    [(b,g), d] layout, DMA out.
"""

import numpy as np
import sys

for _p in ("/opt/trn_rl_repo",):
    if _p not in sys.path:
        sys.path.append(_p)

SCALE = 0.08838834764831845
P = 128  # partition / head-dim / token-tile size


def _seq_order(L):
    """Processing order: big/small alternating. Every small sequence sits
    between two big ones, so its exp->PV dependency latency hides under a
    big sequence's QK phase; ends on the smallest for a short drain."""
    order0 = np.argsort(-np.asarray(L), kind="stable")
    B = len(order0)
    half = (B + 1) // 2
    order = np.empty(B, np.int64)
    order[0::2] = order0[:half]
    order[1::2] = order0[half:]
    return order


def _build_graph(
    nt,
    rem,
    nblk,
    fp8=True,
    dma_only=False,
    pipeline_pv=True,
    replay=1,
    no_dma=False,
    slab=None,
    bufs=10,
    pv_lag=1,
    altq=-1,
    contend=False,
    qk2x=False,
    spbufs=3,
    pvint=True,
    ramp=None,
    fin_chunks=2,
):
    """Build the SPMD Bacc graph, specialized on per-seq tile counts.

    nt[b]  = number of 128-token tiles for seq b (>= 1, includes new token)
    rem[b] = valid tokens in the last tile (1..128)
    nblk   = total 128-token blocks of the staged kv input (sum(nt))
    fp8    = stage K/V as float8 e3m4 (4 mantissa bits): halves HBM traffic
        again vs bf16. The PE multiplies fp8 stationary x bf16 moving
        exactly; q and probs stay bf16, so only the K/V quantization
        (~1.3e-2 rel) enters the error budget.
    dma_only = ablation: issue only the K/V loads (timing the memory floor)
    pipeline_pv = emit seq b's PV phase after seq b+1's score phase, so the
        exp round-trip through ScalarE doesn't stall the PE stream
    """
    import concourse.mybir as mybir
    import concourse.tile as tile
    from concourse import bacc
    from concourse.masks import make_identity

    B = len(nt)
    G = 4  # query heads per core
    MAXNT = int(max(nt))
    off = np.concatenate([[0], np.cumsum(np.asarray(nt, dtype=np.int64))])
    # kv slab boundaries (in 128-token blocks): small slabs at the start so
    # compute begins ~1us in, ~2MB steady-state slabs for peak HBM bandwidth,
    # small slabs at the end to shorten the post-DMA drain.
    SLAB = slab if slab else (64 if fp8 else 32)
    if ramp is not None:
        up, down = [list(r) for r in ramp]
    else:
        up = [max(SLAB // 8, 1), max(SLAB // 4, 1), max(SLAB // 2, 1)]
        down = [max(SLAB // 4, 1), max(SLAB // 8, 1), max(SLAB // 16, 2)]
    mid = nblk - sum(up) - sum(down)
    if mid <= 0:
        sizes = up  # tiny problem: ramp-up only
    else:
        sizes = up + [SLAB] * (mid // SLAB)
        if mid % SLAB:
            sizes.append(mid % SLAB)  # odd slab just before the ramp-down
        sizes += down
    bounds = [0]
    for s in sizes:
        if bounds[-1] >= nblk:
            break
        bounds.append(min(nblk, bounds[-1] + s))
    if bounds[-1] < nblk:
        bounds.append(nblk)
    NS = len(bounds) - 1  # number of kv slabs
    slab_of = np.searchsorted(np.asarray(bounds), np.arange(nblk), side="right") - 1
    f32 = mybir.dt.float32
    bf16 = mybir.dt.bfloat16
    kvdt = mybir.dt.float8e3 if fp8 else bf16

    nc = bacc.Bacc(None, target_bir_lowering=False)
    kv = nc.dram_tensor("kv", [P, nblk * 2 * P], kvdt, kind="ExternalInput")
    qh = nc.dram_tensor("qh", [P, B * G], bf16, kind="ExternalInput")  # [d,(b,g)]
    out = nc.dram_tensor("out", [B, G * P], f32, kind="ExternalOutput")

    with tile.TileContext(nc) as tc:
        with tc.tile_pool(name="persist", bufs=1) as persist:
            ident_f = persist.tile([P, P], f32)
            make_identity(nc, ident_f)
            ones_col_bf = persist.tile([P, 1], bf16)
            nc.vector.memset(ones_col_bf, 1.0)
            ones_row_f = persist.tile([1, P], f32)
            nc.vector.memset(ones_row_f, 1.0)
            # neg_tab[p, r] = 0.0 if p < r else -30.0 — fused into the exp as
            # a per-partition bias for the partial last tile (r = rem):
            # exp(scale*s - 30) ~ 1e-13 zeroes the padding rows without a
            # separate DVE mask op in the PV dependency chain.
            neg_tab = persist.tile([P, P + 1], f32)
            nc.gpsimd.memset(neg_tab, -30.0)
            nc.gpsimd.affine_select(
                out=neg_tab,
                in_=neg_tab,
                compare_op=mybir.AluOpType.is_ge,
                fill=0.0,
                base=0,
                pattern=[[-1, P + 1]],
                channel_multiplier=1,
            )
            qh_bf = persist.tile([P, B * G], bf16)
            nc.gpsimd.dma_start(qh_bf[:], qh[:])
            outT = persist.tile([P, B * G], f32)  # [d, (b,g)]
            l_red = persist.tile([1, B * G], f32)
            recip = persist.tile([1, B * G], f32)

            if no_dma or contend:
                # no_dma=True: one resident dummy slab. no_dma=2: rotate
                # through 8 dummy-slab regions so PE stationary loads hit
                # spread SBUF addresses like the real kernel's rotating pool.
                n_dummy = 8 if no_dma == 2 else 1
                dummy_big = persist.tile([P, n_dummy * SLAB * 2 * P], kvdt)
                W = SLAB * 2 * P
                for j in range(n_dummy):
                    nc.vector.memset(dummy_big[:, j * W : (j + 1) * W], 0.0)
                dummies = [
                    dummy_big[:, j * W : (j + 1) * W] for j in range(n_dummy)
                ]

            with (
                tc.tile_pool(name="kv", bufs=bufs) as kvpool,
                tc.tile_pool(
                    name="sc_ps", bufs=spbufs or pv_lag + 1, space="PSUM"
                ) as scps,
                tc.tile_pool(name="probs", bufs=pv_lag + 1) as prpool,
                tc.tile_pool(name="acc_ps", bufs=3, space="PSUM") as accps,
                tc.tile_pool(name="fin_ps", bufs=1, space="PSUM") as finps,
                tc.tile_pool(name="fin_sb", bufs=2) as fpool,
            ):
                state = {}
                slabs = {}

                dma_engines = [nc.gpsimd, nc.sync, nc.scalar]

                def get_slab(s):
                    """Rotating-pool slab load; emission order is monotone in
                    s because blocks are visited in concat order."""
                    if no_dma or contend:
                        return dummies[s % len(dummies)]
                    if s not in slabs:
                        st = kvpool.tile([P, SLAB * 2 * P], kvdt)
                        lo = bounds[s] * 2 * P
                        hi = bounds[s + 1] * 2 * P
                        if altq == -1:
                            eng = nc.sync
                        elif altq:
                            eng = dma_engines[s % altq]
                        else:
                            eng = nc.gpsimd
                        eng.dma_start(st[:, : hi - lo], kv[:, lo:hi])
                        slabs[s] = st
                    return slabs[s]

                def _kt_of(i):  # K^T [d, t] of global block i
                    s = int(slab_of[i])
                    r = i - bounds[s]
                    return get_slab(s)[:, r * 2 * P : r * 2 * P + P]

                def _vt_of(i):  # V [t, d] of global block i
                    s = int(slab_of[i])
                    r = i - bounds[s]
                    return get_slab(s)[:, r * 2 * P + P : (r + 1) * 2 * P]

                def emit_scores(b, pv_cb=None):
                    ntb = int(nt[b])
                    o = int(off[b])
                    scores = scps.tile([P, G * MAXNT], f32)
                    for _rep in range(2 if qk2x else 1):
                        for i in range(ntb):
                            if (
                                pv_cb is not None
                                and i > 0
                                and slab_of[o + i] != slab_of[o + i - 1]
                            ):
                                # the PE is in-order: place the previous
                                # sequence's (ready) PV work ahead of the
                                # chunks that will wait on the next slab
                                pv_cb()
                                pv_cb = None
                            nc.tensor.matmul(
                                scores[:, G * i : G * (i + 1)],
                                lhsT=_kt_of(o + i),
                                rhs=qh_bf[:, G * b : G * (b + 1)],
                                start=True,
                                stop=True,
                            )
                    pb = prpool.tile([P, G * MAXNT], bf16)
                    r = int(rem[b])
                    full = G * (ntb - 1) if r < P else G * ntb
                    if full:
                        nc.scalar.activation(
                            pb[:, :full],
                            scores[:, :full],
                            mybir.ActivationFunctionType.Exp,
                            scale=SCALE,
                        )
                    if r < P:
                        nc.scalar.activation(
                            pb[:, G * (ntb - 1) : G * ntb],
                            scores[:, G * (ntb - 1) : G * ntb],
                            mybir.ActivationFunctionType.Exp,
                            bias=neg_tab[:, r : r + 1],
                            scale=SCALE,
                        )
                    state[b] = pb

                def emit_pv(b):
                    ntb = int(nt[b])
                    o = int(off[b])
                    pb = state.pop(b)
                    lp = accps.tile([1, G * MAXNT], f32, tag="acc")
                    nc.tensor.matmul(
                        lp[:, : G * ntb],
                        lhsT=ones_col_bf,
                        rhs=pb[:, : G * ntb],
                        start=True,
                        stop=True,
                    )
                    otp = accps.tile([P, G], f32, tag="acc")
                    for i in range(ntb):
                        nc.tensor.matmul(
                            otp,
                            lhsT=_vt_of(o + i),
                            rhs=pb[:, G * i : G * (i + 1)],
                            start=(i == 0),
                            stop=(i == ntb - 1),
                        )
                    nc.vector.tensor_copy(outT[:, G * b : G * (b + 1)], otp)
                    nc.vector.tensor_reduce(
                        l_red[0:1, G * b : G * (b + 1)],
                        lp[0:1, : G * ntb].rearrange("p (i h) -> p h i", h=G),
                        axis=mybir.AxisListType.X,
                        op=mybir.AluOpType.add,
                    )

                out_r = out.rearrange("b (g d) -> (b g) d", g=G)
                fin_state = {}

                def emit_fin1(s0, s1):
                    """Stage 1: 1/l and partition-broadcast (PE dep: recip)."""
                    c0, c1 = G * s0, G * s1
                    n = c1 - c0
                    nc.vector.reciprocal(recip[0:1, c0:c1], l_red[0:1, c0:c1])
                    bc = finps.tile([P, P], f32)
                    nc.tensor.matmul(
                        bc[:, :n],
                        lhsT=ones_row_f,
                        rhs=recip[0:1, c0:c1],
                        start=True,
                        stop=True,
                    )
                    outN = fpool.tile([P, P], f32)
                    nc.vector.tensor_mul(outN[:, :n], outT[:, c0:c1], bc[:, :n])
                    fin_state[s0] = (outN, bc)

                def emit_fin2(s0, s1):
                    """Stage 2: transpose to [(b,g), d] and store (gpsimd queue
                    so the output DMA skips the slab queue's anti-dep waits)."""
                    c0, c1 = G * s0, G * s1
                    n = c1 - c0
                    outN, _ = fin_state.pop(s0)
                    tp2 = finps.tile([P, P], f32)
                    nc.tensor.transpose(tp2[:n, :], outN[:, :n], ident_f)
                    outF = fpool.tile([P, P], f32)
                    nc.vector.tensor_copy(outF[:n, :], tp2[:n, :])
                    nc.gpsimd.dma_start(out_r[c0:c1, :], outF[:n, :])

                def emit_finalize(s0, s1):
                    emit_fin1(s0, s1)
                    emit_fin2(s0, s1)

                def emit_loads_raw():
                    """Real slab loads with tiny consumers (no compute dep)."""
                    for s in range(NS):
                        st = kvpool.tile([P, SLAB * 2 * P], kvdt)
                        lo = bounds[s] * 2 * P
                        hi = bounds[s + 1] * 2 * P
                        if altq == -1:
                            eng = nc.sync
                        elif altq:
                            eng = dma_engines[s % altq]
                        else:
                            eng = nc.gpsimd
                        eng.dma_start(st[:, : hi - lo], kv[:, lo:hi])
                        nc.vector.tensor_copy(outT[0:1, s : s + 1], st[0:1, 0:1])

                def emit_body():
                    slabs.clear()
                    fin_state.clear()
                    if fin_chunks <= 1:
                        fbs = [B]
                    elif fin_chunks == 2:
                        fbs = [3 * B // 4, B]
                    else:
                        fbs = [B * (i + 1) // fin_chunks for i in range(fin_chunks)]
                    # lag each chunk's finalize 2-3 seqs past its boundary so
                    # the PE never head-of-line blocks on the DVE chain
                    events, prev = [], 0
                    for s1 in fbs:
                        events.append((min(s1 + 2, B), 1, prev, s1))
                        events.append((min(s1 + 3, B), 2, prev, s1))
                        prev = s1
                    events.sort(key=lambda e: e[0])
                    done = [0]

                    def run_pv(b):
                        emit_pv(b)
                        done[0] += 1
                        while events and events[0][0] <= done[0]:
                            _, st, s0, s1 = events.pop(0)
                            (emit_fin1 if st == 1 else emit_fin2)(s0, s1)

                    if contend:
                        # real DMA stream + full compute on a dummy slab, no
                        # cross-deps: isolates resource contention from
                        # dependency stalls
                        emit_loads_raw()
                        pend = []
                        for b in range(B):
                            emit_scores(b)
                            pend.append(b)
                            if len(pend) > pv_lag:
                                run_pv(pend.pop(0))
                        for b in pend:
                            run_pv(b)
                    elif dma_only:
                        emit_loads_raw()
                        nc.vector.memset(l_red, 1.0)
                        nc.vector.memset(outT, 0.0)
                        emit_finalize(0, B)
                    elif pipeline_pv:
                        pend = []
                        for b in range(B):
                            cb = None
                            if pvint and len(pend) >= pv_lag:
                                cb = lambda: run_pv(pend.pop(0))
                            emit_scores(b, cb)
                            pend.append(b)
                            if len(pend) > pv_lag:
                                run_pv(pend.pop(0))
                        for b in pend:
                            run_pv(b)
                    else:
                        for b in range(B):
                            emit_scores(b)
                            run_pv(b)

                if replay > 1:
                    with tc.For_i(0, replay, 1):
                        emit_body()
                else:
                    emit_body()
    nc.compile()
    return nc


def _build_probe(nt, rem, nblk, probe, replay=1, slab=64, bufs=6, n_dummy=4):
    """Contention probes with ZERO shared tiles between the DMA stream and the
    compute stream (the old `contend` mode shared outT and the DVE queue,
    serializing the two streams through Tile dependencies).

    probe: 'dma'    = slab loads only, consumers on Pool engine
           'comp'   = full compute mix (QK+exp+PV+DVE) on dummy slabs
           'qk'     = pure PE stream (QK+PV matmuls, dummy probs, no ACT/DVE)
           'both'   = 'dma' + 'comp' concurrently, disjoint resources
           'qk_dma' = 'dma' + 'qk' concurrently, disjoint resources
    """
    import concourse.mybir as mybir
    import concourse.tile as tile
    from concourse import bacc

    B = len(nt)
    G = 4
    MAXNT = int(max(nt))
    off = np.concatenate([[0], np.cumsum(np.asarray(nt, dtype=np.int64))])
    SLAB = slab
    bounds = list(range(0, nblk, SLAB)) + [nblk]
    if bounds[-2] == nblk:
        bounds = bounds[:-1]
    NS = len(bounds) - 1
    f32 = mybir.dt.float32
    bf16 = mybir.dt.bfloat16
    kvdt = mybir.dt.float8e3

    nc = bacc.Bacc(None, target_bir_lowering=False)
    kv = nc.dram_tensor("kv", [P, nblk * 2 * P], kvdt, kind="ExternalInput")
    qh = nc.dram_tensor("qh", [P, B * G], bf16, kind="ExternalInput")
    out = nc.dram_tensor("out", [B, G * P], f32, kind="ExternalOutput")

    do_dma = probe in ("dma", "both", "qk_dma", "qk1_dma", "qk2_dma")
    do_comp = probe in ("comp", "both")
    do_qk = probe in ("qk", "qk_dma")
    do_qk1 = probe in ("qk1", "qk1_dma")  # QK matmuls only (half PE work)
    do_qk2 = probe in ("qk2", "qk2_dma")  # QK emitted twice (double PE work)

    with tile.TileContext(nc) as tc:
        with tc.tile_pool(name="persist", bufs=1) as persist:
            qh_bf = persist.tile([P, B * G], bf16)
            nc.gpsimd.dma_start(qh_bf[:], qh[:])
            outz = persist.tile([B, G * P], f32)
            nc.vector.memset(outz, 0.0)
            sink = persist.tile([1, NS + 1], kvdt)  # Pool consumer target
            neg_tab = persist.tile([P, P + 1], f32)
            nc.gpsimd.memset(neg_tab, -30.0)
            W = SLAB * 2 * P
            dummies = []
            if do_comp or do_qk or do_qk1 or do_qk2:
                dummy_big = persist.tile([P, n_dummy * W], kvdt)
                for j in range(n_dummy):
                    nc.vector.memset(dummy_big[:, j * W : (j + 1) * W], 0.0)
                dummies = [dummy_big[:, j * W : (j + 1) * W] for j in range(n_dummy)]
            pdum = None
            if do_qk or do_qk1 or do_qk2:
                pdum = persist.tile([P, G * MAXNT], bf16)
                nc.vector.memset(pdum, 0.001)
            outT = persist.tile([P, B * G], f32)
            l_red = persist.tile([1, B * G], f32)
            ones_col_bf = persist.tile([P, 1], bf16)
            nc.vector.memset(ones_col_bf, 1.0)

            with (
                tc.tile_pool(name="kv", bufs=bufs) as kvpool,
                tc.tile_pool(name="sc_ps", bufs=3, space="PSUM") as scps,
                tc.tile_pool(name="probs", bufs=2) as prpool,
                tc.tile_pool(name="acc_ps", bufs=3, space="PSUM") as accps,
            ):
                def emit_loads():
                    for s in range(NS):
                        st = kvpool.tile([P, SLAB * 2 * P], kvdt)
                        lo, hi = bounds[s] * 2 * P, bounds[s + 1] * 2 * P
                        nc.sync.dma_start(st[:, : hi - lo], kv[:, lo:hi])
                        nc.gpsimd.tensor_copy(sink[0:1, s : s + 1], st[0:1, 0:1])

                def _kt(i):
                    s, r = divmod(int(i), SLAB)
                    d = dummies[s % n_dummy]
                    return d[:, (r % SLAB) * 2 * P : (r % SLAB) * 2 * P + P]

                def _vt(i):
                    s, r = divmod(int(i), SLAB)
                    d = dummies[s % n_dummy]
                    return d[:, (r % SLAB) * 2 * P + P : ((r % SLAB) + 1) * 2 * P]

                def emit_compute(full):
                    state = {}
                    pend = []
                    for b in range(B):
                        ntb, o = int(nt[b]), int(off[b])
                        scores = scps.tile([P, G * MAXNT], f32)
                        for i in range(ntb):
                            nc.tensor.matmul(
                                scores[:, G * i : G * (i + 1)],
                                lhsT=_kt(o + i),
                                rhs=qh_bf[:, G * b : G * (b + 1)],
                                start=True, stop=True,
                            )
                        if full:
                            pb = prpool.tile([P, G * MAXNT], bf16)
                            nc.scalar.activation(
                                pb[:, : G * ntb], scores[:, : G * ntb],
                                mybir.ActivationFunctionType.Exp, scale=SCALE,
                            )
                            state[b] = pb
                        pend.append(b)
                        if len(pend) > 1:
                            bb = pend.pop(0)
                            nb2, o2 = int(nt[bb]), int(off[bb])
                            pbb = state.pop(bb) if full else pdum
                            otp = accps.tile([P, G], f32, tag="acc")
                            for i in range(nb2):
                                nc.tensor.matmul(
                                    otp, lhsT=_vt(o2 + i),
                                    rhs=pbb[:, G * i : G * (i + 1)],
                                    start=(i == 0), stop=(i == nb2 - 1),
                                )
                            if full:
                                lp = accps.tile([1, G * MAXNT], f32, tag="acc")
                                nc.tensor.matmul(
                                    lp[:, : G * nb2], lhsT=ones_col_bf,
                                    rhs=pbb[:, : G * nb2], start=True, stop=True,
                                )
                                nc.vector.tensor_copy(outT[:, G * bb : G * (bb + 1)], otp)
                                nc.vector.tensor_reduce(
                                    l_red[0:1, G * bb : G * (bb + 1)],
                                    lp[0:1, : G * nb2].rearrange("p (i h) -> p h i", h=G),
                                    axis=mybir.AxisListType.X,
                                    op=mybir.AluOpType.add,
                                )
                            else:
                                nc.vector.tensor_copy(outT[:, G * bb : G * (bb + 1)], otp)
                    for bb in pend:
                        nb2, o2 = int(nt[bb]), int(off[bb])
                        pbb = state.pop(bb) if full else pdum
                        otp = accps.tile([P, G], f32, tag="acc")
                        for i in range(nb2):
                            nc.tensor.matmul(
                                otp, lhsT=_vt(o2 + i),
                                rhs=pbb[:, G * i : G * (i + 1)],
                                start=(i == 0), stop=(i == nb2 - 1),
                            )
                        nc.vector.tensor_copy(outT[:, G * bb : G * (bb + 1)], otp)

                def emit_qk_only():
                    for b in range(B):
                        ntb, o = int(nt[b]), int(off[b])
                        scores = scps.tile([P, G * MAXNT], f32)
                        for i in range(ntb):
                            nc.tensor.matmul(
                                scores[:, G * i : G * (i + 1)],
                                lhsT=_kt(o + i),
                                rhs=qh_bf[:, G * b : G * (b + 1)],
                                start=True, stop=True,
                            )

                def body():
                    if do_dma:
                        emit_loads()
                    if do_comp:
                        emit_compute(full=True)
                    if do_qk:
                        emit_compute(full=False)
                    if do_qk1:
                        emit_qk_only()
                    if do_qk2:
                        emit_compute(full=False)
                        emit_compute(full=False)

                if replay > 1:
                    with tc.For_i(0, replay, 1):
                        body()
                else:
                    body()

            nc.sync.dma_start(out[:, :], outz)
    nc.compile()
    return nc


def _prepare(
    query,
    key,
    value,
    key_cache,
    value_cache,
    block_tables,
    seq_lens,
    build=True,
    fp8=True,
):
    """Build the compiled SPMD graph and the per-core input shards."""
    import ml_dtypes

    bf16 = ml_dtypes.bfloat16
    kvdt = ml_dtypes.float8_e3m4 if fp8 else bf16
    query = np.asarray(query, dtype=np.float32)
    key = np.asarray(key, dtype=np.float32)
    value = np.asarray(value, dtype=np.float32)
    key_cache = np.asarray(key_cache, dtype=np.float32)
    value_cache = np.asarray(value_cache, dtype=np.float32)
    block_tables = np.asarray(block_tables)
    seq_lens = np.asarray(seq_lens)

    B, H, D = query.shape
    KVH = key.shape[1]
    NB, BS = key_cache.shape[0], key_cache.shape[1]
    S_MAX = block_tables.shape[1] * BS
    G = H // KVH
    N_CORES = 8
    assert KVH == N_CORES and D == P

    L = np.maximum(seq_lens.astype(np.int64), 1)
    # `order[s]` = original index of the sequence processed s-th; outputs
    # are unscrambled on the host.
    order = _seq_order(L)
    L = L[order]
    nt = ((L + P - 1) // P).astype(np.int64)  # tiles incl. the new token
    rem = L - (nt - 1) * P  # valid tokens in last tile (1..128)
    off = np.concatenate([[0], np.cumsum(nt)])
    TOT = int(off[-1]) * P

    kc_flat = key_cache.reshape(NB * BS, KVH, D)
    vc_flat = value_cache.reshape(NB * BS, KVH, D)

    # Token slot ids, concatenated per sequence (nt[b]*128 tokens each; the
    # tail past L is read-but-masked padding). With arange block tables (the
    # spec's fill) slot (b, t) is just b*S_MAX + t.
    arange_ok = bool(
        np.array_equal(
            block_tables.ravel(),
            np.arange(block_tables.size, dtype=block_tables.ravel().dtype),
        )
    )
    tok_idx = np.empty(TOT, np.int64)
    for b in range(B):
        ob = int(order[b])  # original sequence index
        t = np.arange(int(nt[b]) * P, dtype=np.int64)
        # tile padding past the sequence's allocated pages re-reads the last
        # valid slot (finite data; zeroed by the exp mask anyway)
        t = np.minimum(t, S_MAX - 1)
        if arange_ok:
            ids = ob * S_MAX + t
        else:
            ids = block_tables[ob, t // BS].astype(np.int64) * BS + t % BS
        tok_idx[off[b] * P : (off[b] + nt[b]) * P] = ids
    newpos = off[:-1] * P + (L - 1)  # new token position in the concat layout

    NBLK = int(off[-1])
    nc = _build_graph(nt, rem, NBLK, fp8=fp8) if build else None

    lim = float(ml_dtypes.finfo(kvdt).max)
    in_maps = []
    for c in range(N_CORES):
        k_sel = kc_flat[tok_idx, c, :]  # [TOT, D] f32
        v_sel = vc_flat[tok_idx, c, :]
        k_sel[newpos] = key[order, c, :]
        v_sel[newpos] = value[order, c, :]
        kt3 = k_sel.T.reshape(P, NBLK, P)  # [d, blk, t]
        vp3 = v_sel.reshape(NBLK, P, P).transpose(1, 0, 2)  # [p, blk, d]
        kv_c = np.ascontiguousarray(
            np.stack([kt3, vp3], axis=2)
            .reshape(P, NBLK * 2 * P)
            .clip(-lim, lim)
            .astype(kvdt)
        )
        qh_c = np.ascontiguousarray(
            query[order][:, c * G : (c + 1) * G, :]
            .transpose(2, 0, 1)
            .reshape(D, B * G)
            .astype(bf16)
        )
        in_maps.append({"kv": kv_c, "qh": qh_c})
    return nc, in_maps, (B, H, D, G), order


def kernel(query, key, value, key_cache, value_cache, block_tables, seq_lens):
    from concourse.bass_utils import run_bass_kernel_spmd

    nc, in_maps, (B, H, D, G), order = _prepare(
        query, key, value, key_cache, value_cache, block_tables, seq_lens
    )
    res = run_bass_kernel_spmd(nc, in_maps, core_ids=list(range(len(in_maps))))
    out = np.empty((B, H * D), np.float32)
    for c in range(len(in_maps)):
        out[order, c * G * D : (c + 1) * G * D] = res.results[c]["out"]
    return out



# revision 20
# speedup vs baseline: 1.1447x; 1.0005x over previous
"""Paged-attention decode (GQA) on 8 Trainium2 NeuronCores.

Sharding: tensor-parallel over heads. Core c owns KV head c (KVH=8) and the
4 query heads in its GQA group. The KV cache is resolved, sliced per-core and
restaged by the host as bf16 (halving HBM read traffic vs f32), with the new
K/V token written in at position L-1 (so the device sees one uniform cache,
no separate new-token path). block_tables and seq_lens are folded into the
compiled graph (decode launch config). Each core runs an identical SPMD graph
with no collectives; the host concatenates the per-core output slices.

Host staging per core c (L = seq_lens[b], nt[b] = ceil(L/128) 128-token
tiles, concatenated across sequences; NBLK = sum(nt)):
  - kv [128, NBLK*256] bf16: per 128-token block i, cols [256i, 256i+128)
    hold K transposed (kv[d, 256i+t] = K[128i+t, d]) and cols
    [256i+128, 256i+256) hold V partition-major (kv[p, 256i+128+d]
    = V[128i+p, d]). K and V interleaved per block so one slab DMA
    (SLAB_BLOCKS*64KB, ~2MB) moves both at near-peak HBM bandwidth.
  - qh [128, B*G] bf16: queries as [d, (b, g)].

Device algorithm per core, per sequence b (tiles i = 0..nt-1):
  - DMA kv slabs (2MB contiguous-per-partition transfers, rotating pool)
  - scores[t, g] per tile: matmul(lhsT=KT_tile [d,t], rhs=q [d,4]) -> PSUM
  - exp(scale*s) on ACT (PSUM -> bf16 SBUF probs); mask tail rows of the
    last tile by a per-partition mask multiply (softmax-without-max:
    scores are O(5), no overflow)
  - out^T[d, 4] += matmul(lhsT=V_tile [t,d], rhs=probs tile [t,4]), PSUM acc
  - denominator l = ones-matmul over probs, reduced per sequence on DVE
  - finalize: broadcast 1/l via a rank-1 matmul, multiply, PE-transpose to
    [(b,g), d] layout, DMA out.
"""

import numpy as np
import sys

for _p in ("/opt/trn_rl_repo",):
    if _p not in sys.path:
        sys.path.append(_p)

SCALE = 0.08838834764831845
P = 128  # partition / head-dim / token-tile size


def _seq_order(L):
    """Processing order: big/small alternating. Every small sequence sits
    between two big ones, so its exp->PV dependency latency hides under a
    big sequence's QK phase; ends on the smallest for a short drain."""
    order0 = np.argsort(-np.asarray(L), kind="stable")
    B = len(order0)
    half = (B + 1) // 2
    order = np.empty(B, np.int64)
    order[0::2] = order0[:half]
    order[1::2] = order0[half:]
    return order


def _build_graph(
    nt,
    rem,
    nblk,
    fp8=True,
    dma_only=False,
    pipeline_pv=True,
    replay=1,
    no_dma=False,
    slab=None,
    bufs=10,
    pv_lag=1,
    altq=-1,
    contend=False,
    qk2x=False,
    spbufs=3,
    pvint=True,
    ramp=None,
    fin_chunks=2,
):
    """Build the SPMD Bacc graph, specialized on per-seq tile counts.

    nt[b]  = number of 128-token tiles for seq b (>= 1, includes new token)
    rem[b] = valid tokens in the last tile (1..128)
    nblk   = total 128-token blocks of the staged kv input (sum(nt))
    fp8    = stage K/V as float8 e3m4 (4 mantissa bits): halves HBM traffic
        again vs bf16. The PE multiplies fp8 stationary x bf16 moving
        exactly; q and probs stay bf16, so only the K/V quantization
        (~1.3e-2 rel) enters the error budget.
    dma_only = ablation: issue only the K/V loads (timing the memory floor)
    pipeline_pv = emit seq b's PV phase after seq b+1's score phase, so the
        exp round-trip through ScalarE doesn't stall the PE stream
    """
    import concourse.mybir as mybir
    import concourse.tile as tile
    from concourse import bacc
    from concourse.masks import make_identity

    B = len(nt)
    G = 4  # query heads per core
    MAXNT = int(max(nt))
    off = np.concatenate([[0], np.cumsum(np.asarray(nt, dtype=np.int64))])
    # kv slab boundaries (in 128-token blocks): small slabs at the start so
    # compute begins ~1us in, ~2MB steady-state slabs for peak HBM bandwidth,
    # small slabs at the end to shorten the post-DMA drain.
    SLAB = slab if slab else (64 if fp8 else 32)
    if ramp is not None:
        up, down = [list(r) for r in ramp]
    else:
        up = [max(SLAB // 8, 1), max(SLAB // 4, 1), max(SLAB // 2, 1)]
        down = [max(SLAB // 4, 1), max(SLAB // 8, 1), max(SLAB // 16, 2)]
    mid = nblk - sum(up) - sum(down)
    if mid <= 0:
        sizes = up  # tiny problem: ramp-up only
    else:
        sizes = up + [SLAB] * (mid // SLAB)
        if mid % SLAB:
            sizes.append(mid % SLAB)  # odd slab just before the ramp-down
        sizes += down
    bounds = [0]
    for s in sizes:
        if bounds[-1] >= nblk:
            break
        bounds.append(min(nblk, bounds[-1] + s))
    if bounds[-1] < nblk:
        bounds.append(nblk)
    NS = len(bounds) - 1  # number of kv slabs
    slab_of = np.searchsorted(np.asarray(bounds), np.arange(nblk), side="right") - 1
    f32 = mybir.dt.float32
    bf16 = mybir.dt.bfloat16
    kvdt = mybir.dt.float8e3 if fp8 else bf16

    nc = bacc.Bacc(None, target_bir_lowering=False)
    kv = nc.dram_tensor("kv", [P, nblk * 2 * P], kvdt, kind="ExternalInput")
    qh = nc.dram_tensor("qh", [P, B * G], bf16, kind="ExternalInput")  # [d,(b,g)]
    out = nc.dram_tensor("out", [B, G * P], f32, kind="ExternalOutput")

    with tile.TileContext(nc) as tc:
        with tc.tile_pool(name="persist", bufs=1) as persist:
            ident_f = persist.tile([P, P], f32)
            make_identity(nc, ident_f)
            ones_col_bf = persist.tile([P, 1], bf16)
            nc.vector.memset(ones_col_bf, 1.0)
            ones_row_f = persist.tile([1, P], f32)
            nc.vector.memset(ones_row_f, 1.0)
            # neg_tab[p, r] = 0.0 if p < r else -30.0 — fused into the exp as
            # a per-partition bias for the partial last tile (r = rem):
            # exp(scale*s - 30) ~ 1e-13 zeroes the padding rows without a
            # separate DVE mask op in the PV dependency chain.
            neg_tab = persist.tile([P, P + 1], f32)
            nc.gpsimd.memset(neg_tab, -30.0)
            nc.gpsimd.affine_select(
                out=neg_tab,
                in_=neg_tab,
                compare_op=mybir.AluOpType.is_ge,
                fill=0.0,
                base=0,
                pattern=[[-1, P + 1]],
                channel_multiplier=1,
            )
            qh_bf = persist.tile([P, B * G], bf16)
            nc.gpsimd.dma_start(qh_bf[:], qh[:])
            outT = persist.tile([P, B * G], f32)  # [d, (b,g)]
            l_red = persist.tile([1, B * G], f32)
            recip = persist.tile([1, B * G], f32)

            if no_dma or contend:
                # no_dma=True: one resident dummy slab. no_dma=2: rotate
                # through 8 dummy-slab regions so PE stationary loads hit
                # spread SBUF addresses like the real kernel's rotating pool.
                n_dummy = 8 if no_dma == 2 else 1
                dummy_big = persist.tile([P, n_dummy * SLAB * 2 * P], kvdt)
                W = SLAB * 2 * P
                for j in range(n_dummy):
                    nc.vector.memset(dummy_big[:, j * W : (j + 1) * W], 0.0)
                dummies = [
                    dummy_big[:, j * W : (j + 1) * W] for j in range(n_dummy)
                ]

            with (
                tc.tile_pool(name="kv", bufs=bufs) as kvpool,
                tc.tile_pool(
                    name="sc_ps", bufs=spbufs or pv_lag + 1, space="PSUM"
                ) as scps,
                tc.tile_pool(name="probs", bufs=pv_lag + 1) as prpool,
                tc.tile_pool(name="acc_ps", bufs=3, space="PSUM") as accps,
                tc.tile_pool(name="fin_ps", bufs=1, space="PSUM") as finps,
                tc.tile_pool(name="fin_sb", bufs=2) as fpool,
            ):
                state = {}
                slabs = {}

                dma_engines = [nc.gpsimd, nc.sync, nc.scalar]

                def get_slab(s):
                    """Rotating-pool slab load; emission order is monotone in
                    s because blocks are visited in concat order."""
                    if no_dma or contend:
                        return dummies[s % len(dummies)]
                    if s not in slabs:
                        st = kvpool.tile([P, SLAB * 2 * P], kvdt)
                        lo = bounds[s] * 2 * P
                        hi = bounds[s + 1] * 2 * P
                        if altq == -1:
                            eng = nc.sync
                        elif altq:
                            eng = dma_engines[s % altq]
                        else:
                            eng = nc.gpsimd
                        eng.dma_start(st[:, : hi - lo], kv[:, lo:hi])
                        slabs[s] = st
                    return slabs[s]

                def _kt_of(i):  # K^T [d, t] of global block i
                    s = int(slab_of[i])
                    r = i - bounds[s]
                    return get_slab(s)[:, r * 2 * P : r * 2 * P + P]

                def _vt_of(i):  # V [t, d] of global block i
                    s = int(slab_of[i])
                    r = i - bounds[s]
                    return get_slab(s)[:, r * 2 * P + P : (r + 1) * 2 * P]

                def emit_scores(b, pv_cb=None):
                    ntb = int(nt[b])
                    o = int(off[b])
                    scores = scps.tile([P, G * MAXNT], f32)
                    for _rep in range(2 if qk2x else 1):
                        for i in range(ntb):
                            if (
                                pv_cb is not None
                                and i > 0
                                and slab_of[o + i] != slab_of[o + i - 1]
                            ):
                                # the PE is in-order: place the previous
                                # sequence's (ready) PV work ahead of the
                                # chunks that will wait on the next slab
                                pv_cb()
                                pv_cb = None
                            nc.tensor.matmul(
                                scores[:, G * i : G * (i + 1)],
                                lhsT=_kt_of(o + i),
                                rhs=qh_bf[:, G * b : G * (b + 1)],
                                start=True,
                                stop=True,
                            )
                    pb = prpool.tile([P, G * MAXNT], bf16)
                    r = int(rem[b])
                    full = G * (ntb - 1) if r < P else G * ntb
                    if full:
                        nc.scalar.activation(
                            pb[:, :full],
                            scores[:, :full],
                            mybir.ActivationFunctionType.Exp,
                            scale=SCALE,
                        )
                    if r < P:
                        nc.scalar.activation(
                            pb[:, G * (ntb - 1) : G * ntb],
                            scores[:, G * (ntb - 1) : G * ntb],
                            mybir.ActivationFunctionType.Exp,
                            bias=neg_tab[:, r : r + 1],
                            scale=SCALE,
                        )
                    state[b] = pb

                def emit_pv(b):
                    ntb = int(nt[b])
                    o = int(off[b])
                    pb = state.pop(b)
                    lp = accps.tile([1, G * MAXNT], f32, tag="acc")
                    nc.tensor.matmul(
                        lp[:, : G * ntb],
                        lhsT=ones_col_bf,
                        rhs=pb[:, : G * ntb],
                        start=True,
                        stop=True,
                    )
                    otp = accps.tile([P, G], f32, tag="acc")
                    for i in range(ntb):
                        nc.tensor.matmul(
                            otp,
                            lhsT=_vt_of(o + i),
                            rhs=pb[:, G * i : G * (i + 1)],
                            start=(i == 0),
                            stop=(i == ntb - 1),
                        )
                    nc.vector.tensor_copy(outT[:, G * b : G * (b + 1)], otp)
                    nc.vector.tensor_reduce(
                        l_red[0:1, G * b : G * (b + 1)],
                        lp[0:1, : G * ntb].rearrange("p (i h) -> p h i", h=G),
                        axis=mybir.AxisListType.X,
                        op=mybir.AluOpType.add,
                    )

                out_r = out.rearrange("b (g d) -> (b g) d", g=G)
                fin_state = {}

                def emit_fin1(s0, s1):
                    """Stage 1: 1/l and partition-broadcast (PE dep: recip)."""
                    c0, c1 = G * s0, G * s1
                    n = c1 - c0
                    nc.vector.reciprocal(recip[0:1, c0:c1], l_red[0:1, c0:c1])
                    bc = finps.tile([P, P], f32)
                    nc.tensor.matmul(
                        bc[:, :n],
                        lhsT=ones_row_f,
                        rhs=recip[0:1, c0:c1],
                        start=True,
                        stop=True,
                    )
                    outN = fpool.tile([P, P], f32)
                    nc.vector.tensor_mul(outN[:, :n], outT[:, c0:c1], bc[:, :n])
                    fin_state[s0] = (outN, bc)

                def emit_fin2(s0, s1):
                    """Stage 2: transpose to [(b,g), d] and store. Mid-stream
                    chunks DMA via gpsimd (skips the slab queue's anti-dep
                    waits); the last chunk via sync (HWDGE's ~3x lower fixed
                    cost shortens the exposed tail — slab queue is empty by
                    then)."""
                    c0, c1 = G * s0, G * s1
                    n = c1 - c0
                    outN, _ = fin_state.pop(s0)
                    tp2 = finps.tile([P, P], f32)
                    nc.tensor.transpose(tp2[:n, :], outN[:, :n], ident_f)
                    outF = fpool.tile([P, P], f32)
                    nc.vector.tensor_copy(outF[:n, :], tp2[:n, :])
                    eng = nc.sync if s1 >= B else nc.gpsimd
                    eng.dma_start(out_r[c0:c1, :], outF[:n, :])

                def emit_finalize(s0, s1):
                    emit_fin1(s0, s1)
                    emit_fin2(s0, s1)

                def emit_loads_raw():
                    """Real slab loads with tiny consumers (no compute dep)."""
                    for s in range(NS):
                        st = kvpool.tile([P, SLAB * 2 * P], kvdt)
                        lo = bounds[s] * 2 * P
                        hi = bounds[s + 1] * 2 * P
                        if altq == -1:
                            eng = nc.sync
                        elif altq:
                            eng = dma_engines[s % altq]
                        else:
                            eng = nc.gpsimd
                        eng.dma_start(st[:, : hi - lo], kv[:, lo:hi])
                        nc.vector.tensor_copy(outT[0:1, s : s + 1], st[0:1, 0:1])

                def emit_body():
                    slabs.clear()
                    fin_state.clear()
                    if fin_chunks <= 1:
                        fbs = [B]
                    elif fin_chunks == 2:
                        fbs = [3 * B // 4, B]
                    else:
                        fbs = [B * (i + 1) // fin_chunks for i in range(fin_chunks)]
                    # lag each chunk's finalize 2-3 seqs past its boundary so
                    # the PE never head-of-line blocks on the DVE chain
                    events, prev = [], 0
                    for s1 in fbs:
                        events.append((min(s1 + 2, B), 1, prev, s1))
                        events.append((min(s1 + 3, B), 2, prev, s1))
                        prev = s1
                    events.sort(key=lambda e: e[0])
                    done = [0]

                    def run_pv(b):
                        emit_pv(b)
                        done[0] += 1
                        while events and events[0][0] <= done[0]:
                            _, st, s0, s1 = events.pop(0)
                            (emit_fin1 if st == 1 else emit_fin2)(s0, s1)

                    if contend:
                        # real DMA stream + full compute on a dummy slab, no
                        # cross-deps: isolates resource contention from
                        # dependency stalls
                        emit_loads_raw()
                        pend = []
                        for b in range(B):
                            emit_scores(b)
                            pend.append(b)
                            if len(pend) > pv_lag:
                                run_pv(pend.pop(0))
                        for b in pend:
                            run_pv(b)
                    elif dma_only:
                        emit_loads_raw()
                        nc.vector.memset(l_red, 1.0)
                        nc.vector.memset(outT, 0.0)
                        emit_finalize(0, B)
                    elif pipeline_pv:
                        pend = []
                        for b in range(B):
                            cb = None
                            if pvint and len(pend) >= pv_lag:
                                cb = lambda: run_pv(pend.pop(0))
                            emit_scores(b, cb)
                            pend.append(b)
                            if len(pend) > pv_lag:
                                run_pv(pend.pop(0))
                        for b in pend:
                            run_pv(b)
                    else:
                        for b in range(B):
                            emit_scores(b)
                            run_pv(b)

                if replay > 1:
                    with tc.For_i(0, replay, 1):
                        emit_body()
                else:
                    emit_body()
    nc.compile()
    return nc


def _build_probe(nt, rem, nblk, probe, replay=1, slab=64, bufs=6, n_dummy=4):
    """Contention probes with ZERO shared tiles between the DMA stream and the
    compute stream (the old `contend` mode shared outT and the DVE queue,
    serializing the two streams through Tile dependencies).

    probe: 'dma'    = slab loads only, consumers on Pool engine
           'comp'   = full compute mix (QK+exp+PV+DVE) on dummy slabs
           'qk'     = pure PE stream (QK+PV matmuls, dummy probs, no ACT/DVE)
           'both'   = 'dma' + 'comp' concurrently, disjoint resources
           'qk_dma' = 'dma' + 'qk' concurrently, disjoint resources
    """
    import concourse.mybir as mybir
    import concourse.tile as tile
    from concourse import bacc

    B = len(nt)
    G = 4
    MAXNT = int(max(nt))
    off = np.concatenate([[0], np.cumsum(np.asarray(nt, dtype=np.int64))])
    SLAB = slab
    bounds = list(range(0, nblk, SLAB)) + [nblk]
    if bounds[-2] == nblk:
        bounds = bounds[:-1]
    NS = len(bounds) - 1
    f32 = mybir.dt.float32
    bf16 = mybir.dt.bfloat16
    kvdt = mybir.dt.float8e3

    nc = bacc.Bacc(None, target_bir_lowering=False)
    kv = nc.dram_tensor("kv", [P, nblk * 2 * P], kvdt, kind="ExternalInput")
    qh = nc.dram_tensor("qh", [P, B * G], bf16, kind="ExternalInput")
    out = nc.dram_tensor("out", [B, G * P], f32, kind="ExternalOutput")

    do_dma = probe in ("dma", "both", "qk_dma", "qk1_dma", "qk2_dma")
    do_comp = probe in ("comp", "both")
    do_qk = probe in ("qk", "qk_dma")
    do_qk1 = probe in ("qk1", "qk1_dma")  # QK matmuls only (half PE work)
    do_qk2 = probe in ("qk2", "qk2_dma")  # QK emitted twice (double PE work)

    with tile.TileContext(nc) as tc:
        with tc.tile_pool(name="persist", bufs=1) as persist:
            qh_bf = persist.tile([P, B * G], bf16)
            nc.gpsimd.dma_start(qh_bf[:], qh[:])
            outz = persist.tile([B, G * P], f32)
            nc.vector.memset(outz, 0.0)
            sink = persist.tile([1, NS + 1], kvdt)  # Pool consumer target
            neg_tab = persist.tile([P, P + 1], f32)
            nc.gpsimd.memset(neg_tab, -30.0)
            W = SLAB * 2 * P
            dummies = []
            if do_comp or do_qk or do_qk1 or do_qk2:
                dummy_big = persist.tile([P, n_dummy * W], kvdt)
                for j in range(n_dummy):
                    nc.vector.memset(dummy_big[:, j * W : (j + 1) * W], 0.0)
                dummies = [dummy_big[:, j * W : (j + 1) * W] for j in range(n_dummy)]
            pdum = None
            if do_qk or do_qk1 or do_qk2:
                pdum = persist.tile([P, G * MAXNT], bf16)
                nc.vector.memset(pdum, 0.001)
            outT = persist.tile([P, B * G], f32)
            l_red = persist.tile([1, B * G], f32)
            ones_col_bf = persist.tile([P, 1], bf16)
            nc.vector.memset(ones_col_bf, 1.0)

            with (
                tc.tile_pool(name="kv", bufs=bufs) as kvpool,
                tc.tile_pool(name="sc_ps", bufs=3, space="PSUM") as scps,
                tc.tile_pool(name="probs", bufs=2) as prpool,
                tc.tile_pool(name="acc_ps", bufs=3, space="PSUM") as accps,
            ):
                def emit_loads():
                    for s in range(NS):
                        st = kvpool.tile([P, SLAB * 2 * P], kvdt)
                        lo, hi = bounds[s] * 2 * P, bounds[s + 1] * 2 * P
                        nc.sync.dma_start(st[:, : hi - lo], kv[:, lo:hi])
                        nc.gpsimd.tensor_copy(sink[0:1, s : s + 1], st[0:1, 0:1])

                def _kt(i):
                    s, r = divmod(int(i), SLAB)
                    d = dummies[s % n_dummy]
                    return d[:, (r % SLAB) * 2 * P : (r % SLAB) * 2 * P + P]

                def _vt(i):
                    s, r = divmod(int(i), SLAB)
                    d = dummies[s % n_dummy]
                    return d[:, (r % SLAB) * 2 * P + P : ((r % SLAB) + 1) * 2 * P]

                def emit_compute(full):
                    state = {}
                    pend = []
                    for b in range(B):
                        ntb, o = int(nt[b]), int(off[b])
                        scores = scps.tile([P, G * MAXNT], f32)
                        for i in range(ntb):
                            nc.tensor.matmul(
                                scores[:, G * i : G * (i + 1)],
                                lhsT=_kt(o + i),
                                rhs=qh_bf[:, G * b : G * (b + 1)],
                                start=True, stop=True,
                            )
                        if full:
                            pb = prpool.tile([P, G * MAXNT], bf16)
                            nc.scalar.activation(
                                pb[:, : G * ntb], scores[:, : G * ntb],
                                mybir.ActivationFunctionType.Exp, scale=SCALE,
                            )
                            state[b] = pb
                        pend.append(b)
                        if len(pend) > 1:
                            bb = pend.pop(0)
                            nb2, o2 = int(nt[bb]), int(off[bb])
                            pbb = state.pop(bb) if full else pdum
                            otp = accps.tile([P, G], f32, tag="acc")
                            for i in range(nb2):
                                nc.tensor.matmul(
                                    otp, lhsT=_vt(o2 + i),
                                    rhs=pbb[:, G * i : G * (i + 1)],
                                    start=(i == 0), stop=(i == nb2 - 1),
                                )
                            if full:
                                lp = accps.tile([1, G * MAXNT], f32, tag="acc")
                                nc.tensor.matmul(
                                    lp[:, : G * nb2], lhsT=ones_col_bf,
                                    rhs=pbb[:, : G * nb2], start=True, stop=True,
                                )
                                nc.vector.tensor_copy(outT[:, G * bb : G * (bb + 1)], otp)
                                nc.vector.tensor_reduce(
                                    l_red[0:1, G * bb : G * (bb + 1)],
                                    lp[0:1, : G * nb2].rearrange("p (i h) -> p h i", h=G),
                                    axis=mybir.AxisListType.X,
                                    op=mybir.AluOpType.add,
                                )
                            else:
                                nc.vector.tensor_copy(outT[:, G * bb : G * (bb + 1)], otp)
                    for bb in pend:
                        nb2, o2 = int(nt[bb]), int(off[bb])
                        pbb = state.pop(bb) if full else pdum
                        otp = accps.tile([P, G], f32, tag="acc")
                        for i in range(nb2):
                            nc.tensor.matmul(
                                otp, lhsT=_vt(o2 + i),
                                rhs=pbb[:, G * i : G * (i + 1)],
                                start=(i == 0), stop=(i == nb2 - 1),
                            )
                        nc.vector.tensor_copy(outT[:, G * bb : G * (bb + 1)], otp)

                def emit_qk_only():
                    for b in range(B):
                        ntb, o = int(nt[b]), int(off[b])
                        scores = scps.tile([P, G * MAXNT], f32)
                        for i in range(ntb):
                            nc.tensor.matmul(
                                scores[:, G * i : G * (i + 1)],
                                lhsT=_kt(o + i),
                                rhs=qh_bf[:, G * b : G * (b + 1)],
                                start=True, stop=True,
                            )

                def body():
                    if do_dma:
                        emit_loads()
                    if do_comp:
                        emit_compute(full=True)
                    if do_qk:
                        emit_compute(full=False)
                    if do_qk1:
                        emit_qk_only()
                    if do_qk2:
                        emit_compute(full=False)
                        emit_compute(full=False)

                if replay > 1:
                    with tc.For_i(0, replay, 1):
                        body()
                else:
                    body()

            nc.sync.dma_start(out[:, :], outz)
    nc.compile()
    return nc


def _prepare(
    query,
    key,
    value,
    key_cache,
    value_cache,
    block_tables,
    seq_lens,
    build=True,
    fp8=True,
):
    """Build the compiled SPMD graph and the per-core input shards."""
    import ml_dtypes

    bf16 = ml_dtypes.bfloat16
    kvdt = ml_dtypes.float8_e3m4 if fp8 else bf16
    query = np.asarray(query, dtype=np.float32)
    key = np.asarray(key, dtype=np.float32)
    value = np.asarray(value, dtype=np.float32)
    key_cache = np.asarray(key_cache, dtype=np.float32)
    value_cache = np.asarray(value_cache, dtype=np.float32)
    block_tables = np.asarray(block_tables)
    seq_lens = np.asarray(seq_lens)

    B, H, D = query.shape
    KVH = key.shape[1]
    NB, BS = key_cache.shape[0], key_cache.shape[1]
    S_MAX = block_tables.shape[1] * BS
    G = H // KVH
    N_CORES = 8
    assert KVH == N_CORES and D == P

    L = np.maximum(seq_lens.astype(np.int64), 1)
    # `order[s]` = original index of the sequence processed s-th; outputs
    # are unscrambled on the host.
    order = _seq_order(L)
    L = L[order]
    nt = ((L + P - 1) // P).astype(np.int64)  # tiles incl. the new token
    rem = L - (nt - 1) * P  # valid tokens in last tile (1..128)
    off = np.concatenate([[0], np.cumsum(nt)])
    TOT = int(off[-1]) * P

    kc_flat = key_cache.reshape(NB * BS, KVH, D)
    vc_flat = value_cache.reshape(NB * BS, KVH, D)

    # Token slot ids, concatenated per sequence (nt[b]*128 tokens each; the
    # tail past L is read-but-masked padding). With arange block tables (the
    # spec's fill) slot (b, t) is just b*S_MAX + t.
    arange_ok = bool(
        np.array_equal(
            block_tables.ravel(),
            np.arange(block_tables.size, dtype=block_tables.ravel().dtype),
        )
    )
    tok_idx = np.empty(TOT, np.int64)
    for b in range(B):
        ob = int(order[b])  # original sequence index
        t = np.arange(int(nt[b]) * P, dtype=np.int64)
        # tile padding past the sequence's allocated pages re-reads the last
        # valid slot (finite data; zeroed by the exp mask anyway)
        t = np.minimum(t, S_MAX - 1)
        if arange_ok:
            ids = ob * S_MAX + t
        else:
            ids = block_tables[ob, t // BS].astype(np.int64) * BS + t % BS
        tok_idx[off[b] * P : (off[b] + nt[b]) * P] = ids
    newpos = off[:-1] * P + (L - 1)  # new token position in the concat layout

    NBLK = int(off[-1])
    nc = _build_graph(nt, rem, NBLK, fp8=fp8) if build else None

    lim = float(ml_dtypes.finfo(kvdt).max)
    in_maps = []
    for c in range(N_CORES):
        k_sel = kc_flat[tok_idx, c, :]  # [TOT, D] f32
        v_sel = vc_flat[tok_idx, c, :]
        k_sel[newpos] = key[order, c, :]
        v_sel[newpos] = value[order, c, :]
        kt3 = k_sel.T.reshape(P, NBLK, P)  # [d, blk, t]
        vp3 = v_sel.reshape(NBLK, P, P).transpose(1, 0, 2)  # [p, blk, d]
        kv_c = np.ascontiguousarray(
            np.stack([kt3, vp3], axis=2)
            .reshape(P, NBLK * 2 * P)
            .clip(-lim, lim)
            .astype(kvdt)
        )
        qh_c = np.ascontiguousarray(
            query[order][:, c * G : (c + 1) * G, :]
            .transpose(2, 0, 1)
            .reshape(D, B * G)
            .astype(bf16)
        )
        in_maps.append({"kv": kv_c, "qh": qh_c})
    return nc, in_maps, (B, H, D, G), order


def kernel(query, key, value, key_cache, value_cache, block_tables, seq_lens):
    from concourse.bass_utils import run_bass_kernel_spmd

    nc, in_maps, (B, H, D, G), order = _prepare(
        query, key, value, key_cache, value_cache, block_tables, seq_lens
    )
    res = run_bass_kernel_spmd(nc, in_maps, core_ids=list(range(len(in_maps))))
    out = np.empty((B, H * D), np.float32)
    for c in range(len(in_maps)):
        out[order, c * G * D : (c + 1) * G * D] = res.results[c]["out"]
    return out



# revision 32
# speedup vs baseline: 1.3356x; 1.1668x over previous
"""Paged-attention decode (GQA) on 8 Trainium2 NeuronCores.

Sharding: tensor-parallel over heads. Core c owns KV head c (KVH=8) and the
4 query heads in its GQA group. The KV cache is resolved, sliced per-core and
restaged by the host as bf16 (halving HBM read traffic vs f32), with the new
K/V token written in at position L-1 (so the device sees one uniform cache,
no separate new-token path). block_tables and seq_lens are folded into the
compiled graph (decode launch config). Each core runs an identical SPMD graph
with no collectives; the host concatenates the per-core output slices.

Host staging per core c (L = seq_lens[b], nt[b] = ceil(L/128) 128-token
tiles, concatenated across sequences; NBLK = sum(nt)):
  - kv [128, NBLK*256] bf16: per 128-token block i, cols [256i, 256i+128)
    hold K transposed (kv[d, 256i+t] = K[128i+t, d]) and cols
    [256i+128, 256i+256) hold V partition-major (kv[p, 256i+128+d]
    = V[128i+p, d]). K and V interleaved per block so one slab DMA
    (SLAB_BLOCKS*64KB, ~2MB) moves both at near-peak HBM bandwidth.
  - qh [128, B*G] bf16: queries as [d, (b, g)].

Device algorithm per core, per sequence b (tiles i = 0..nt-1):
  - DMA kv slabs (2MB contiguous-per-partition transfers, rotating pool)
  - scores[t, g] per tile: matmul(lhsT=KT_tile [d,t], rhs=q [d,4]) -> PSUM
  - exp(scale*s) on ACT (PSUM -> bf16 SBUF probs); mask tail rows of the
    last tile by a per-partition mask multiply (softmax-without-max:
    scores are O(5), no overflow)
  - out^T[d, 4] += matmul(lhsT=V_tile [t,d], rhs=probs tile [t,4]), PSUM acc
  - denominator l = ones-matmul over probs, reduced per sequence on DVE
  - finalize: broadcast 1/l via a rank-1 matmul, multiply, PE-transpose to
    [(b,g), d] layout, DMA out.
"""

import numpy as np
import sys

for _p in ("/opt/trn_rl_repo",):
    if _p not in sys.path:
        sys.path.append(_p)

SCALE = 0.08838834764831845
P = 128  # partition / head-dim / token-tile size


def _seq_order(L):
    """Processing order: big/small alternating. Every small sequence sits
    between two big ones, so its exp->PV dependency latency hides under a
    big sequence's QK phase; ends on the smallest for a short drain."""
    order0 = np.argsort(-np.asarray(L), kind="stable")
    B = len(order0)
    half = (B + 1) // 2
    order = np.empty(B, np.int64)
    order[0::2] = order0[:half]
    order[1::2] = order0[half:]
    return order


def _build_graph(
    nt,
    rem,
    nblk,
    fp8=True,
    dma_only=False,
    pipeline_pv=True,
    replay=1,
    no_dma=False,
    slab=None,
    bufs=10,
    pv_lag=1,
    altq=-1,
    contend=False,
    qk2x=False,
    spbufs=3,
    pvint=True,
    ramp=None,
    fin_chunks=2,
    pb_fp8=False,  # fp8 probs NaN: e3m4 range too narrow for unmaxed
    # softmax, and e4m3 moving x e3m4 stationary doesn't decode correctly
):
    """Build the SPMD Bacc graph, specialized on per-seq tile counts.

    nt[b]  = number of 128-token tiles for seq b (>= 1, includes new token)
    rem[b] = valid tokens in the last tile (1..128)
    nblk   = total 128-token blocks of the staged kv input (sum(nt))
    fp8    = stage K/V as float8 e3m4 (4 mantissa bits): halves HBM traffic
        again vs bf16. The PE multiplies fp8 stationary x bf16 moving
        exactly; q and probs stay bf16, so only the K/V quantization
        (~1.3e-2 rel) enters the error budget.
    dma_only = ablation: issue only the K/V loads (timing the memory floor)
    pipeline_pv = emit seq b's PV phase after seq b+1's score phase, so the
        exp round-trip through ScalarE doesn't stall the PE stream
    """
    import concourse.mybir as mybir
    import concourse.tile as tile
    from concourse import bacc
    from concourse.masks import make_identity

    B = len(nt)
    G = 4  # query heads per core
    MAXNT = int(max(nt))
    off = np.concatenate([[0], np.cumsum(np.asarray(nt, dtype=np.int64))])
    # kv slab boundaries (in 128-token blocks): small slabs at the start so
    # compute begins ~1us in, ~2MB steady-state slabs for peak HBM bandwidth,
    # small slabs at the end to shorten the post-DMA drain.
    SLAB = slab if slab else (64 if fp8 else 32)
    if ramp is not None:
        up, down = [list(r) for r in ramp]
    else:
        up = [
            max(SLAB // 16, 2),
            max(SLAB // 8, 1),
            max(SLAB // 4, 1),
            max(SLAB // 2, 1),
        ]
        down = [max(SLAB // 4, 1), max(SLAB // 8, 1), max(SLAB // 16, 2)]
    mid = nblk - sum(up) - sum(down)
    if mid <= 0:
        sizes = up  # tiny problem: ramp-up only
    else:
        sizes = up + [SLAB] * (mid // SLAB)
        if mid % SLAB:
            sizes.append(mid % SLAB)  # odd slab just before the ramp-down
        sizes += down
    bounds = [0]
    for s in sizes:
        if bounds[-1] >= nblk:
            break
        bounds.append(min(nblk, bounds[-1] + s))
    if bounds[-1] < nblk:
        bounds.append(nblk)
    NS = len(bounds) - 1  # number of kv slabs
    slab_of = np.searchsorted(np.asarray(bounds), np.arange(nblk), side="right") - 1
    f32 = mybir.dt.float32
    bf16 = mybir.dt.bfloat16
    kvdt = mybir.dt.float8e3 if fp8 else bf16
    # probs in fp8 halve pb SBUF traffic. e3m4 matches the V tiles' format
    # (one fp8 decode per matmul); exp gets a -2.5 bias so probs fit e3m4's
    # +/-15.9 range — a uniform exp factor cancels in the softmax ratio.
    if pb_fp8 == "e3":
        pbdt, pb_bias = mybir.dt.float8e3, -2.5
    elif pb_fp8:
        pbdt, pb_bias = mybir.dt.float8e4, 0.0
    else:
        pbdt, pb_bias = bf16, 0.0

    nc = bacc.Bacc(None, target_bir_lowering=False)
    kv = nc.dram_tensor("kv", [P, nblk * 2 * P], kvdt, kind="ExternalInput")
    qh = nc.dram_tensor("qh", [P, B * G], bf16, kind="ExternalInput")  # [d,(b,g)]
    out = nc.dram_tensor("out", [B, G * P], f32, kind="ExternalOutput")

    with tile.TileContext(nc) as tc:
        with tc.tile_pool(name="persist", bufs=1) as persist:
            ident_f = persist.tile([P, P], f32)
            make_identity(nc, ident_f)
            ones_col_bf = persist.tile([P, 1], bf16)
            nc.vector.memset(ones_col_bf, 1.0)
            ones_row_f = persist.tile([1, P], f32)
            nc.vector.memset(ones_row_f, 1.0)
            # neg_tab[p, r] = 0.0 if p < r else -30.0 — fused into the exp as
            # a per-partition bias for the partial last tile (r = rem):
            # exp(scale*s - 30) ~ 1e-13 zeroes the padding rows without a
            # separate DVE mask op in the PV dependency chain.
            neg_tab = persist.tile([P, P + 1], f32)
            nc.gpsimd.memset(neg_tab, -30.0 + pb_bias)
            nc.gpsimd.affine_select(
                out=neg_tab,
                in_=neg_tab,
                compare_op=mybir.AluOpType.is_ge,
                fill=pb_bias,
                base=0,
                pattern=[[-1, P + 1]],
                channel_multiplier=1,
            )
            # scalar queue (HWDGE): low fixed cost, parallel to sync's slab0,
            # so q lands before the first QK needs it
            qh_bf = persist.tile([P, B * G], bf16)
            nc.scalar.dma_start(qh_bf[:], qh[:])
            outT = persist.tile([P, B * G], f32)  # [d, (b,g)]
            l_red = persist.tile([1, B * G], f32)
            recip = persist.tile([1, B * G], f32)

            if no_dma or contend:
                # no_dma=True: one resident dummy slab. no_dma=2: rotate
                # through 8 dummy-slab regions so PE stationary loads hit
                # spread SBUF addresses like the real kernel's rotating pool.
                n_dummy = 8 if no_dma == 2 else 1
                dummy_big = persist.tile([P, n_dummy * SLAB * 2 * P], kvdt)
                W = SLAB * 2 * P
                for j in range(n_dummy):
                    nc.vector.memset(dummy_big[:, j * W : (j + 1) * W], 0.0)
                dummies = [
                    dummy_big[:, j * W : (j + 1) * W] for j in range(n_dummy)
                ]

            with (
                tc.tile_pool(name="kv", bufs=bufs) as kvpool,
                tc.tile_pool(
                    name="sc_ps", bufs=spbufs or pv_lag + 1, space="PSUM"
                ) as scps,
                tc.tile_pool(name="probs", bufs=pv_lag + 1) as prpool,
                tc.tile_pool(name="acc_ps", bufs=3, space="PSUM") as accps,
                tc.tile_pool(name="fin_ps", bufs=1, space="PSUM") as finps,
                tc.tile_pool(name="fin_sb", bufs=2) as fpool,
            ):
                state = {}
                slabs = {}

                dma_engines = [nc.gpsimd, nc.sync, nc.scalar]

                def get_slab(s):
                    """Rotating-pool slab load; emission order is monotone in
                    s because blocks are visited in concat order."""
                    if no_dma or contend:
                        return dummies[s % len(dummies)]
                    if s not in slabs:
                        st = kvpool.tile([P, SLAB * 2 * P], kvdt)
                        lo = bounds[s] * 2 * P
                        hi = bounds[s + 1] * 2 * P
                        if altq == -1:
                            eng = nc.sync
                        elif altq:
                            eng = dma_engines[s % altq]
                        else:
                            eng = nc.gpsimd
                        eng.dma_start(st[:, : hi - lo], kv[:, lo:hi])
                        slabs[s] = st
                    return slabs[s]

                def _kt_of(i):  # K^T [d, t] of global block i
                    s = int(slab_of[i])
                    r = i - bounds[s]
                    return get_slab(s)[:, r * 2 * P : r * 2 * P + P]

                def _vt_of(i):  # V [t, d] of global block i
                    s = int(slab_of[i])
                    r = i - bounds[s]
                    return get_slab(s)[:, r * 2 * P + P : (r + 1) * 2 * P]

                def emit_scores(b, pv_cb=None):
                    ntb = int(nt[b])
                    o = int(off[b])
                    scores = scps.tile([P, G * MAXNT], f32)
                    for _rep in range(2 if qk2x else 1):
                        for i in range(ntb):
                            if (
                                pv_cb is not None
                                and i > 0
                                and slab_of[o + i] != slab_of[o + i - 1]
                            ):
                                # the PE is in-order: place the previous
                                # sequence's (ready) PV work ahead of the
                                # chunks that will wait on the next slab
                                pv_cb()
                                pv_cb = None
                            nc.tensor.matmul(
                                scores[:, G * i : G * (i + 1)],
                                lhsT=_kt_of(o + i),
                                rhs=qh_bf[:, G * b : G * (b + 1)],
                                start=True,
                                stop=True,
                            )
                    pb = prpool.tile([P, G * MAXNT], pbdt)
                    r = int(rem[b])
                    full = G * (ntb - 1) if r < P else G * ntb
                    if full:
                        nc.scalar.activation(
                            pb[:, :full],
                            scores[:, :full],
                            mybir.ActivationFunctionType.Exp,
                            # col P of neg_tab is constant pb_bias (r=P row
                            # threshold covers every partition)
                            bias=neg_tab[:, P : P + 1] if pb_bias else 0.0,
                            scale=SCALE,
                        )
                    if r < P:
                        nc.scalar.activation(
                            pb[:, G * (ntb - 1) : G * ntb],
                            scores[:, G * (ntb - 1) : G * ntb],
                            mybir.ActivationFunctionType.Exp,
                            bias=neg_tab[:, r : r + 1],
                            scale=SCALE,
                        )
                    state[b] = pb

                def emit_pv(b):
                    ntb = int(nt[b])
                    o = int(off[b])
                    pb = state.pop(b)
                    lp = accps.tile([1, G * MAXNT], f32, tag="acc")
                    nc.tensor.matmul(
                        lp[:, : G * ntb],
                        lhsT=ones_col_bf,
                        rhs=pb[:, : G * ntb],
                        start=True,
                        stop=True,
                    )
                    otp = accps.tile([P, G], f32, tag="acc")
                    for i in range(ntb):
                        nc.tensor.matmul(
                            otp,
                            lhsT=_vt_of(o + i),
                            rhs=pb[:, G * i : G * (i + 1)],
                            start=(i == 0),
                            stop=(i == ntb - 1),
                        )
                    nc.vector.tensor_copy(outT[:, G * b : G * (b + 1)], otp)
                    nc.vector.tensor_reduce(
                        l_red[0:1, G * b : G * (b + 1)],
                        lp[0:1, : G * ntb].rearrange("p (i h) -> p h i", h=G),
                        axis=mybir.AxisListType.X,
                        op=mybir.AluOpType.add,
                    )

                out_r = out.rearrange("b (g d) -> (b g) d", g=G)
                fin_state = {}

                def emit_fin1(s0, s1):
                    """Stage 1: 1/l and partition-broadcast (PE dep: recip)."""
                    c0, c1 = G * s0, G * s1
                    n = c1 - c0
                    nc.vector.reciprocal(recip[0:1, c0:c1], l_red[0:1, c0:c1])
                    bc = finps.tile([P, P], f32)
                    nc.tensor.matmul(
                        bc[:, :n],
                        lhsT=ones_row_f,
                        rhs=recip[0:1, c0:c1],
                        start=True,
                        stop=True,
                    )
                    outN = fpool.tile([P, P], f32)
                    nc.vector.tensor_mul(outN[:, :n], outT[:, c0:c1], bc[:, :n])
                    fin_state[s0] = (outN, bc)

                def emit_fin2(s0, s1):
                    """Stage 2: transpose to [(b,g), d] and store. Mid-stream
                    chunks DMA via gpsimd (skips the slab queue's anti-dep
                    waits); the last chunk via sync (HWDGE's ~3x lower fixed
                    cost shortens the exposed tail — slab queue is empty by
                    then)."""
                    c0, c1 = G * s0, G * s1
                    n = c1 - c0
                    outN, _ = fin_state.pop(s0)
                    tp2 = finps.tile([P, P], f32)
                    nc.tensor.transpose(tp2[:n, :], outN[:, :n], ident_f)
                    outF = fpool.tile([P, P], f32)
                    nc.vector.tensor_copy(outF[:n, :], tp2[:n, :])
                    eng = nc.sync if s1 >= B - 1 else nc.gpsimd
                    eng.dma_start(out_r[c0:c1, :], outF[:n, :])

                def emit_finalize(s0, s1):
                    emit_fin1(s0, s1)
                    emit_fin2(s0, s1)

                def emit_loads_raw():
                    """Real slab loads with tiny consumers (no compute dep)."""
                    for s in range(NS):
                        st = kvpool.tile([P, SLAB * 2 * P], kvdt)
                        lo = bounds[s] * 2 * P
                        hi = bounds[s + 1] * 2 * P
                        if altq == -1:
                            eng = nc.sync
                        elif altq:
                            eng = dma_engines[s % altq]
                        else:
                            eng = nc.gpsimd
                        eng.dma_start(st[:, : hi - lo], kv[:, lo:hi])
                        nc.vector.tensor_copy(outT[0:1, s : s + 1], st[0:1, 0:1])

                def emit_body():
                    slabs.clear()
                    fin_state.clear()
                    # lag each chunk's finalize 2-3 seqs past its boundary so
                    # the PE never head-of-line blocks on the DVE chain; the
                    # tail is a 3-way split — bulk chunk hidden mid-stream,
                    # then [3B/4, B-1) one seq early, then a 1-seq drain.
                    if fin_chunks == 2 and B >= 8:
                        q = 3 * B // 4
                        events = [
                            (min(q + 2, B), 1, 0, q),
                            (min(q + 3, B), 2, 0, q),
                            (B - 1, 1, q, B - 1),
                            (B, 2, q, B - 1),
                            (B, 1, B - 1, B),
                            (B, 2, B - 1, B),
                        ]
                    else:
                        if fin_chunks <= 1:
                            fbs = [B]
                        else:
                            fbs = [
                                B * (i + 1) // fin_chunks
                                for i in range(fin_chunks)
                            ]
                        events, prev = [], 0
                        for s1 in fbs:
                            events.append((min(s1 + 2, B), 1, prev, s1))
                            events.append((min(s1 + 3, B), 2, prev, s1))
                            prev = s1
                    events.sort(key=lambda e: e[0])
                    done = [0]

                    def run_pv(b):
                        emit_pv(b)
                        done[0] += 1
                        while events and events[0][0] <= done[0]:
                            _, st, s0, s1 = events.pop(0)
                            (emit_fin1 if st == 1 else emit_fin2)(s0, s1)

                    if contend:
                        # real DMA stream + full compute on a dummy slab, no
                        # cross-deps: isolates resource contention from
                        # dependency stalls
                        emit_loads_raw()
                        pend = []
                        for b in range(B):
                            emit_scores(b)
                            pend.append(b)
                            if len(pend) > pv_lag:
                                run_pv(pend.pop(0))
                        for b in pend:
                            run_pv(b)
                    elif dma_only:
                        emit_loads_raw()
                        nc.vector.memset(l_red, 1.0)
                        nc.vector.memset(outT, 0.0)
                        emit_finalize(0, B)
                    elif pipeline_pv:
                        pend = []
                        for b in range(B):
                            cb = None
                            if pvint and len(pend) >= pv_lag:
                                cb = lambda: run_pv(pend.pop(0))
                            emit_scores(b, cb)
                            pend.append(b)
                            if len(pend) > pv_lag:
                                run_pv(pend.pop(0))
                        for b in pend:
                            run_pv(b)
                    else:
                        for b in range(B):
                            emit_scores(b)
                            run_pv(b)

                if replay > 1:
                    with tc.For_i(0, replay, 1):
                        emit_body()
                else:
                    emit_body()
    nc.compile()
    return nc


def _build_probe(nt, rem, nblk, probe, replay=1, slab=64, bufs=6, n_dummy=4):
    """Contention probes with ZERO shared tiles between the DMA stream and the
    compute stream (the old `contend` mode shared outT and the DVE queue,
    serializing the two streams through Tile dependencies).

    probe: 'dma'    = slab loads only, consumers on Pool engine
           'comp'   = full compute mix (QK+exp+PV+DVE) on dummy slabs
           'qk'     = pure PE stream (QK+PV matmuls, dummy probs, no ACT/DVE)
           'both'   = 'dma' + 'comp' concurrently, disjoint resources
           'qk_dma' = 'dma' + 'qk' concurrently, disjoint resources
    """
    import concourse.mybir as mybir
    import concourse.tile as tile
    from concourse import bacc

    B = len(nt)
    G = 4
    MAXNT = int(max(nt))
    off = np.concatenate([[0], np.cumsum(np.asarray(nt, dtype=np.int64))])
    SLAB = slab
    bounds = list(range(0, nblk, SLAB)) + [nblk]
    if bounds[-2] == nblk:
        bounds = bounds[:-1]
    NS = len(bounds) - 1
    f32 = mybir.dt.float32
    bf16 = mybir.dt.bfloat16
    kvdt = mybir.dt.float8e3

    nc = bacc.Bacc(None, target_bir_lowering=False)
    kv = nc.dram_tensor("kv", [P, nblk * 2 * P], kvdt, kind="ExternalInput")
    qh = nc.dram_tensor("qh", [P, B * G], bf16, kind="ExternalInput")
    out = nc.dram_tensor("out", [B, G * P], f32, kind="ExternalOutput")

    do_dma = probe in ("dma", "both", "qk_dma", "qk1_dma", "qk2_dma")
    do_comp = probe in ("comp", "both")
    do_qk = probe in ("qk", "qk_dma")
    do_qk1 = probe in ("qk1", "qk1_dma")  # QK matmuls only (half PE work)
    do_qk2 = probe in ("qk2", "qk2_dma")  # QK emitted twice (double PE work)

    with tile.TileContext(nc) as tc:
        with tc.tile_pool(name="persist", bufs=1) as persist:
            qh_bf = persist.tile([P, B * G], bf16)
            nc.gpsimd.dma_start(qh_bf[:], qh[:])
            outz = persist.tile([B, G * P], f32)
            nc.vector.memset(outz, 0.0)
            sink = persist.tile([1, NS + 1], kvdt)  # Pool consumer target
            neg_tab = persist.tile([P, P + 1], f32)
            nc.gpsimd.memset(neg_tab, -30.0)
            W = SLAB * 2 * P
            dummies = []
            if do_comp or do_qk or do_qk1 or do_qk2:
                dummy_big = persist.tile([P, n_dummy * W], kvdt)
                for j in range(n_dummy):
                    nc.vector.memset(dummy_big[:, j * W : (j + 1) * W], 0.0)
                dummies = [dummy_big[:, j * W : (j + 1) * W] for j in range(n_dummy)]
            pdum = None
            if do_qk or do_qk1 or do_qk2:
                pdum = persist.tile([P, G * MAXNT], bf16)
                nc.vector.memset(pdum, 0.001)
            outT = persist.tile([P, B * G], f32)
            l_red = persist.tile([1, B * G], f32)
            ones_col_bf = persist.tile([P, 1], bf16)
            nc.vector.memset(ones_col_bf, 1.0)

            with (
                tc.tile_pool(name="kv", bufs=bufs) as kvpool,
                tc.tile_pool(name="sc_ps", bufs=3, space="PSUM") as scps,
                tc.tile_pool(name="probs", bufs=2) as prpool,
                tc.tile_pool(name="acc_ps", bufs=3, space="PSUM") as accps,
            ):
                def emit_loads():
                    for s in range(NS):
                        st = kvpool.tile([P, SLAB * 2 * P], kvdt)
                        lo, hi = bounds[s] * 2 * P, bounds[s + 1] * 2 * P
                        nc.sync.dma_start(st[:, : hi - lo], kv[:, lo:hi])
                        nc.gpsimd.tensor_copy(sink[0:1, s : s + 1], st[0:1, 0:1])

                def _kt(i):
                    s, r = divmod(int(i), SLAB)
                    d = dummies[s % n_dummy]
                    return d[:, (r % SLAB) * 2 * P : (r % SLAB) * 2 * P + P]

                def _vt(i):
                    s, r = divmod(int(i), SLAB)
                    d = dummies[s % n_dummy]
                    return d[:, (r % SLAB) * 2 * P + P : ((r % SLAB) + 1) * 2 * P]

                def emit_compute(full):
                    state = {}
                    pend = []
                    for b in range(B):
                        ntb, o = int(nt[b]), int(off[b])
                        scores = scps.tile([P, G * MAXNT], f32)
                        for i in range(ntb):
                            nc.tensor.matmul(
                                scores[:, G * i : G * (i + 1)],
                                lhsT=_kt(o + i),
                                rhs=qh_bf[:, G * b : G * (b + 1)],
                                start=True, stop=True,
                            )
                        if full:
                            pb = prpool.tile([P, G * MAXNT], bf16)
                            nc.scalar.activation(
                                pb[:, : G * ntb], scores[:, : G * ntb],
                                mybir.ActivationFunctionType.Exp, scale=SCALE,
                            )
                            state[b] = pb
                        pend.append(b)
                        if len(pend) > 1:
                            bb = pend.pop(0)
                            nb2, o2 = int(nt[bb]), int(off[bb])
                            pbb = state.pop(bb) if full else pdum
                            otp = accps.tile([P, G], f32, tag="acc")
                            for i in range(nb2):
                                nc.tensor.matmul(
                                    otp, lhsT=_vt(o2 + i),
                                    rhs=pbb[:, G * i : G * (i + 1)],
                                    start=(i == 0), stop=(i == nb2 - 1),
                                )
                            if full:
                                lp = accps.tile([1, G * MAXNT], f32, tag="acc")
                                nc.tensor.matmul(
                                    lp[:, : G * nb2], lhsT=ones_col_bf,
                                    rhs=pbb[:, : G * nb2], start=True, stop=True,
                                )
                                nc.vector.tensor_copy(outT[:, G * bb : G * (bb + 1)], otp)
                                nc.vector.tensor_reduce(
                                    l_red[0:1, G * bb : G * (bb + 1)],
                                    lp[0:1, : G * nb2].rearrange("p (i h) -> p h i", h=G),
                                    axis=mybir.AxisListType.X,
                                    op=mybir.AluOpType.add,
                                )
                            else:
                                nc.vector.tensor_copy(outT[:, G * bb : G * (bb + 1)], otp)
                    for bb in pend:
                        nb2, o2 = int(nt[bb]), int(off[bb])
                        pbb = state.pop(bb) if full else pdum
                        otp = accps.tile([P, G], f32, tag="acc")
                        for i in range(nb2):
                            nc.tensor.matmul(
                                otp, lhsT=_vt(o2 + i),
                                rhs=pbb[:, G * i : G * (i + 1)],
                                start=(i == 0), stop=(i == nb2 - 1),
                            )
                        nc.vector.tensor_copy(outT[:, G * bb : G * (bb + 1)], otp)

                def emit_qk_only():
                    for b in range(B):
                        ntb, o = int(nt[b]), int(off[b])
                        scores = scps.tile([P, G * MAXNT], f32)
                        for i in range(ntb):
                            nc.tensor.matmul(
                                scores[:, G * i : G * (i + 1)],
                                lhsT=_kt(o + i),
                                rhs=qh_bf[:, G * b : G * (b + 1)],
                                start=True, stop=True,
                            )

                def body():
                    if do_dma:
                        emit_loads()
                    if do_comp:
                        emit_compute(full=True)
                    if do_qk:
                        emit_compute(full=False)
                    if do_qk1:
                        emit_qk_only()
                    if do_qk2:
                        emit_compute(full=False)
                        emit_compute(full=False)

                if replay > 1:
                    with tc.For_i(0, replay, 1):
                        body()
                else:
                    body()

            nc.sync.dma_start(out[:, :], outz)
    nc.compile()
    return nc


def _prepare(
    query,
    key,
    value,
    key_cache,
    value_cache,
    block_tables,
    seq_lens,
    build=True,
    fp8=True,
):
    """Build the compiled SPMD graph and the per-core input shards."""
    import ml_dtypes

    bf16 = ml_dtypes.bfloat16
    kvdt = ml_dtypes.float8_e3m4 if fp8 else bf16
    query = np.asarray(query, dtype=np.float32)
    key = np.asarray(key, dtype=np.float32)
    value = np.asarray(value, dtype=np.float32)
    key_cache = np.asarray(key_cache, dtype=np.float32)
    value_cache = np.asarray(value_cache, dtype=np.float32)
    block_tables = np.asarray(block_tables)
    seq_lens = np.asarray(seq_lens)

    B, H, D = query.shape
    KVH = key.shape[1]
    NB, BS = key_cache.shape[0], key_cache.shape[1]
    S_MAX = block_tables.shape[1] * BS
    G = H // KVH
    N_CORES = 8
    assert KVH == N_CORES and D == P

    L = np.maximum(seq_lens.astype(np.int64), 1)
    # `order[s]` = original index of the sequence processed s-th; outputs
    # are unscrambled on the host.
    order = _seq_order(L)
    L = L[order]
    nt = ((L + P - 1) // P).astype(np.int64)  # tiles incl. the new token
    rem = L - (nt - 1) * P  # valid tokens in last tile (1..128)
    off = np.concatenate([[0], np.cumsum(nt)])
    TOT = int(off[-1]) * P

    kc_flat = key_cache.reshape(NB * BS, KVH, D)
    vc_flat = value_cache.reshape(NB * BS, KVH, D)

    # Token slot ids, concatenated per sequence (nt[b]*128 tokens each; the
    # tail past L is read-but-masked padding). With arange block tables (the
    # spec's fill) slot (b, t) is just b*S_MAX + t.
    arange_ok = bool(
        np.array_equal(
            block_tables.ravel(),
            np.arange(block_tables.size, dtype=block_tables.ravel().dtype),
        )
    )
    tok_idx = np.empty(TOT, np.int64)
    for b in range(B):
        ob = int(order[b])  # original sequence index
        t = np.arange(int(nt[b]) * P, dtype=np.int64)
        # tile padding past the sequence's allocated pages re-reads the last
        # valid slot (finite data; zeroed by the exp mask anyway)
        t = np.minimum(t, S_MAX - 1)
        if arange_ok:
            ids = ob * S_MAX + t
        else:
            ids = block_tables[ob, t // BS].astype(np.int64) * BS + t % BS
        tok_idx[off[b] * P : (off[b] + nt[b]) * P] = ids
    newpos = off[:-1] * P + (L - 1)  # new token position in the concat layout

    NBLK = int(off[-1])
    nc = _build_graph(nt, rem, NBLK, fp8=fp8) if build else None

    lim = float(ml_dtypes.finfo(kvdt).max)
    in_maps = []
    for c in range(N_CORES):
        k_sel = kc_flat[tok_idx, c, :]  # [TOT, D] f32
        v_sel = vc_flat[tok_idx, c, :]
        k_sel[newpos] = key[order, c, :]
        v_sel[newpos] = value[order, c, :]
        kt3 = k_sel.T.reshape(P, NBLK, P)  # [d, blk, t]
        vp3 = v_sel.reshape(NBLK, P, P).transpose(1, 0, 2)  # [p, blk, d]
        kv_c = np.ascontiguousarray(
            np.stack([kt3, vp3], axis=2)
            .reshape(P, NBLK * 2 * P)
            .clip(-lim, lim)
            .astype(kvdt)
        )
        qh_c = np.ascontiguousarray(
            query[order][:, c * G : (c + 1) * G, :]
            .transpose(2, 0, 1)
            .reshape(D, B * G)
            .astype(bf16)
        )
        in_maps.append({"kv": kv_c, "qh": qh_c})
    return nc, in_maps, (B, H, D, G), order


def kernel(query, key, value, key_cache, value_cache, block_tables, seq_lens):
    from concourse.bass_utils import run_bass_kernel_spmd

    nc, in_maps, (B, H, D, G), order = _prepare(
        query, key, value, key_cache, value_cache, block_tables, seq_lens
    )
    res = run_bass_kernel_spmd(nc, in_maps, core_ids=list(range(len(in_maps))))
    out = np.empty((B, H * D), np.float32)
    for c in range(len(in_maps)):
        out[order, c * G * D : (c + 1) * G * D] = res.results[c]["out"]
    return out

